# revision 88
# baseline (speedup 1.0000x reference)
"""Multi-head attention kernel for 8 Trainium2 NeuronCores.

Problem: B=4, T=2048, DIM=1024, 16 heads, head_dim=64, additive causal mask.
  q,k,v = x@W{q,k,v}.T ; attn = softmax(q k^T/8 + mask) ; out = (attn v)@Wo.T

Sharding (no collectives): core i handles batch i//2 and head-group i%2
(8 heads).  Each core projects q/k/v for its 8 heads only (512 features,
no duplicated projection work), runs full causal attention for those heads,
and computes a partial output projection (contraction over its 512
features).  The host sums the two partial outputs per batch while
unsharding.

On-chip math:
 - Q/K projections run as fp8(e4m3) DoubleRow matmuls (K=256 per instr,
   0.5 cyc/row) with a hi+lo 3-term split (x_hi*w_hi + x_hi*w_lo +
   x_lo*w_hi) for near-bf16 accuracy at 2x bf16 speed; x hi/lo planes
   are interleaved in one SBUF tile.  The V projection uses the 3-term
   split only for the first key chunk(s): later rows average many keys,
   so dropping x*wv_lo there is ~0.6% rms and invisible in max error.
 - Scores S[k,q] accumulate in PSUM fp32 (bf16 operands); exp via the
   scalar engine with scale=1/(8*32*32); causal handled by skipping
   fully-masked 128-blocks and one tril-mask multiply per diagonal
   block (on GPSIMD, keeping DVE free).
 - AV uses P as the stationary operand and [V|1] as the moving operand
   (65 cols), giving the softmax denominator for free in column 64.
   Attention output is normalized on DVE, block-transposed [q,hd] ->
   [hd,q] on the DMA XBAR (14ns/16x128 tile; the LAST pair uses PE
   transposes instead - the XBAR round trip would sit on the tail), and
   split into fp8 hi/lo for the output projection.
 - The output projection runs fp8 DR 3-term over (otn, wo) hi/lo pairs
   with feature pairs r-interleaved (K=256/instr): 25% cheaper than
   bf16.  All scales (x32 on wq/wk/wv/wo) fold into the exp scale and a
   1/1024 factor on the final PSUM evacuation.

Scheduling: per-(head-pair, query-group) chunk loops; pairs run in a
STAIRCASE order over (fc, qg) so the exp-heavy large query groups start
early and hide the front-loaded projection filler.  Each key chunk's
scores for both heads share one [128,1024] PSUM tile and ONE exp
instruction; the next chunk's S matmuls are emitted one chunk ahead of
filler and AV work so the scalar engine's feed never queues behind
them.  Projection tiles act as PE filler, emitted as generators at
single-matmul granularity and woven between exp and the dependent AVs
(due-slot hints; NO bulk boundary drains).  Correctness of emission
order - the tile dep-tracker orders by emission, so a consumer emitted
before its producer would read garbage - is enforced by ensure() tags:
S forces its kt/qt tiles, AV forces its va tile, o-groups gate on a
per-token-group hi/lo counter.  Each pair's tail (last AVs + normalize
+ transpose) is carried into the next pair after its first S/exp.  The
final four token blocks' output projections are split into phase A
(fc0/1 terms, runs during the last pair's transpose+hi/lo) and phase B
across all 8 PSUM banks.  PSUM: 2x[128,1024] S + 2x[128,260] per-head
accumulators + 2x[128,512] projection tiles = 8 banks; each
accumulator uses one start/stop per PSUM bank (hardware zeroes whole
banks).  DMAs are few and large (HWDGE issue is ~650ns serial), with
the startup chain split by hi/lo plane so kt0/qt0 begin ASAP.
"""

import sys
import numpy as np

sys.path.insert(0, "/opt/trn_rl_repo")

import ml_dtypes  # noqa: E402
from contextlib import ExitStack  # noqa: E402
from concourse import bass, bacc, tile  # noqa: E402
from concourse.bass_utils import run_bass_kernel_spmd  # noqa: E402
from concourse.masks import make_identity  # noqa: E402

mybir = bass.mybir

B, T, DIM, H, HD = 4, 2048, 1024, 16, 64
HL = 8             # heads per core (head-group)
FG = 512           # features per core (HL * HD)
NJ = 4             # DoubleRow contraction chunks (256 each)
NTC = T // 128     # 16 key chunks
BF16 = mybir.dt.bfloat16
F8 = mybir.dt.float8e4
F32 = mybir.dt.float32
WS = 32.0          # weight pre-scale for e4m3
f8 = ml_dtypes.float8_e4m3fn
bf = ml_dtypes.bfloat16


# --------------------------------------------------------------------------
# fast causal path
# --------------------------------------------------------------------------

V_TERMS = 3       # 2: v = x @ wv_hi (drops the wv_lo correction); 3: full

# schedule tuning knobs (overridden by the sweep harness)
TUNE = {
    "pump_early": 12,   # in-loop pump chunks for positions <= 1 (0: off)
    "pump_fast": 10,    # in-loop pump chunks when a drain backlog exists
    "pump_base": 3,     # in-loop pump chunks otherwise
    "drain_cap": 0,     # boundary drain step cap
    "pump_last": 99,    # pump cap on each pair's final chunk
    "pair_order": None,  # override (fc, qg) processing order
    "vc3": 1,           # key chunks with full 3-term V projection
    "warmup": 0,        # dummy PE matmuls at t=0 (p-state ramp resets on
                        # idle in the cost model, so this is off)
    "og_delay": 0,      # extra positions to defer o-group fillers by
}


def _build_fast(reps=1):
    nc = bacc.Bacc("TRN2", target_bir_lowering=False, debug=False, num_devices=8)
    DR = mybir.MatmulPerfMode.DoubleRow
    Exp = mybir.ActivationFunctionType.Exp
    mult = mybir.AluOpType.mult
    sub = mybir.AluOpType.subtract

    # x hi/lo interleaved: [p, j, i, h, t]; contraction d = 256j + 128i + p
    xhl_t = nc.dram_tensor("xhl", [128, NJ * 2 * 2 * T], F8, kind="ExternalInput").ap()
    # wq/wk hi+lo merged per weight: [h, (j p), (i f)]
    w_t = {}
    for w in ("wq", "wk"):
        w_t[w] = nc.dram_tensor(w + "8", [2, 512, 1024], F8, kind="ExternalInput").ap()
    w_t["wv"] = nc.dram_tensor(
        "wv8", [V_TERMS - 1, 512, 1024], F8, kind="ExternalInput"
    ).ap()
    # wo hi/lo in DR layout: [p, a, r, o]; feature f = 256a + 128r + p
    wo8h_t = nc.dram_tensor("wo8h", [128, 2 * 2 * DIM], F8, kind="ExternalInput").ap()
    wo8l_t = nc.dram_tensor("wo8l", [128, 2 * 2 * DIM], F8, kind="ExternalInput").ap()
    em_t = nc.dram_tensor("em", [128, 128], BF16, kind="ExternalInput").ap()
    y_t = nc.dram_tensor("y", [T, DIM], BF16, kind="ExternalOutput").ap()

    with tile.TileContext(nc) as tc:
      for _rep in range(reps):
        ctx = ExitStack()
        ctx.__enter__()

        # ---- pools -------------------------------------------------------
        x_p = ctx.enter_context(tc.tile_pool(name="xp", bufs=1))
        w_p = ctx.enter_context(tc.tile_pool(name="wp", bufs=1))
        wo_p = ctx.enter_context(tc.tile_pool(name="wop", bufs=1))
        kt_p = ctx.enter_context(tc.tile_pool(name="ktp", bufs=1))
        qt_p = ctx.enter_context(tc.tile_pool(name="qtp", bufs=1))
        va_p = ctx.enter_context(tc.tile_pool(name="vap", bufs=1))
        misc_p = ctx.enter_context(tc.tile_pool(name="miscp", bufs=1))
        otn_p = ctx.enter_context(tc.tile_pool(name="otnp", bufs=1))
        p_p = ctx.enter_context(tc.tile_pool(name="pp", bufs=6))
        nt_p = ctx.enter_context(tc.tile_pool(name="ntp", bufs=3))
        rec_p = ctx.enter_context(tc.tile_pool(name="recp", bufs=4))
        fin_p = ctx.enter_context(tc.tile_pool(name="finp", bufs=4))
        # PSUM: s 2x[128,1024] = 4 banks, o 2x[128,260] = 2 (shared with the
        # [128,128] transpose outputs), w 2x[128,512] = 2  -> 8 banks.
        psS = ctx.enter_context(tc.tile_pool(name="psS", bufs=2, space="PSUM"))
        psO = ctx.enter_context(tc.tile_pool(name="psO", bufs=2, space="PSUM"))
        psW = ctx.enter_context(tc.tile_pool(name="psW", bufs=2, space="PSUM"))

        # ---- persistent SBUF tiles --------------------------------------
        x_hl = x_p.tile([128, NJ, 2, 2, T], F8, tag="xhl", name="x_hl")
        w_sb = {}
        for w in ("wq", "wk"):
            w_sb[w] = w_p.tile([128, 2, NJ, 2, FG], F8, tag=w, name=w)
        w_sb["wv"] = w_p.tile(
            [128, V_TERMS - 1, NJ, 2, FG], F8, tag="wv", name="wv"
        )
        # wo / otn fp8 DR tiles: [p, a, r, ...]; feature f = 256a + 128r + p
        wo8h = wo_p.tile([128, 2, 2, DIM], F8, tag="wo8h", name="wo8h")
        wo8l = wo_p.tile([128, 2, 2, DIM], F8, tag="wo8l", name="wo8l")
        kt_sb = [kt_p.tile([128, T], BF16, tag=f"kt{i}", name=f"kt{i}") for i in range(4)]
        qt_sb = [qt_p.tile([128, T], BF16, tag=f"qt{i}", name=f"qt{i}") for i in range(4)]
        va_sb = [va_p.tile([128, HL, 65], BF16, tag=f"va{i}", name=f"va{i}") for i in range(NTC)]
        otnB = otn_p.tile([128, 2, 2, 16, 128], BF16, tag="otnB", name="otnB")
        otn8h = otn_p.tile([128, 2, 2, 16, 128], F8, tag="otn8h", name="otn8h")
        otn8l = otn_p.tile([128, 2, 2, 16, 128], F8, tag="otn8l", name="otn8l")
        em_sb = misc_p.tile([128, 128], BF16, tag="em", name="em_sb")
        ident = misc_p.tile([128, 128], BF16, tag="id", name="ident")
        make_identity(nc, ident[:])

        # PE warm-up: the tensor engine runs at 1/2-1/4 clock until it has
        # been continuously busy ~3us.  Burn that ramp on dummy matmuls
        # over scratch data while the first input DMAs are still in
        # flight, so the real projections run at full clock.
        if TUNE["warmup"]:
            wm_sb = misc_p.tile([128, 512], BF16, tag="wm", name="wm_sb")
            nc.vector.memset(wm_sb[:], 1.0)
            wm_ps = psW.tile([128, 512], F32, tag="w", name="wm_ps")
            n = TUNE["warmup"]
            for i in range(n):
                nc.tensor.matmul(
                    wm_ps[:], ident[:], wm_sb[:],
                    start=(i == 0), stop=(i == n - 1),
                )
            nc.vector.tensor_copy(wm_sb[:], wm_ps[:])

        # ---- input DMAs --------------------------------------------------
        # HWDGE serializes DMA issue (~650ns each) and transfers serialize on
        # the DMA engines, so strict first-needed order; the startup chain
        # (wk/x/wq for kt0+qt0) is split by hi/lo plane to unblock the first
        # projection matmuls as early as possible.
        def dma_w(key, h):
            nc.sync.dma_start(
                w_sb[key][:, h].rearrange("p a b c -> p a (b c)"),
                w_t[key][h].rearrange("(j p) f -> p j f", j=NJ),
            )

        def dma_wv():
            nc.sync.dma_start(
                w_sb["wv"][:].rearrange("p h a b c -> p h a (b c)"),
                w_t["wv"].rearrange("h (j p) f -> p h j f", j=NJ),
            )

        def dma_x(lo, hi, h=None):
            src = xhl_t[:].rearrange("p (j i h t) -> p j i h t", j=NJ, i=2, h=2)
            if h is None:
                nc.sync.dma_start(x_hl[:, :, :, :, lo:hi], src[:, :, :, :, lo:hi])
            else:
                nc.sync.dma_start(
                    x_hl[:, :, :, h, lo:hi], src[:, :, :, h, lo:hi]
                )

        dma_w("wk", 0)
        dma_x(0, 512, 0)
        dma_w("wk", 1)
        dma_x(0, 512, 1)
        dma_w("wq", 0)
        dma_w("wq", 1)
        dma_wv()
        nc.sync.dma_start(em_sb[:], em_t[:])
        dma_x(512, 1024)
        nc.sync.dma_start(
            wo8h[:], wo8h_t[:].rearrange("p (a r o) -> p a r o", a=2, r=2)
        )
        nc.sync.dma_start(
            wo8l[:], wo8l_t[:].rearrange("p (a r o) -> p a r o", a=2, r=2)
        )
        dma_x(1024, 1536)
        dma_x(1536, 2048)

        for c in range(NTC):
            nc.vector.memset(va_sb[c][:, :, 64:65], 1.0)

        # ---- projection tile emitters (generators yielding every matmul
        # so filler can be woven at fine granularity) ----------------------
        # term order (x h-idx, w h-idx): x-lo last, so the startup tiles can
        # begin before the x-lo plane's DMA lands
        KQ_TERMS = ((0, 0), (0, 1), (1, 0))
        VV_TERMS = KQ_TERMS if V_TERMS == 3 else ((0, 0), (1, 0))
        # early key chunks keep the full 3-term V projection: early-row
        # attention averages few keys, so the wv_lo correction matters
        # there; later rows average it away (error ~3.6%/sqrt(n_eff)).
        VC3 = TUNE.get("vc3", 2)

        def kq_tile_gen(w, dst, fc, tg):
            """K or Q projection tile: out [128 feat, 512 tok]."""
            ps = psW.tile([128, 512], F32, tag="w", name=f"{w}_ps")
            n = 3 * NJ
            i = 0
            for (xh, wh) in KQ_TERMS:
                for j in range(NJ):
                    nc.tensor.matmul(
                        ps[:],
                        w_sb[w][:, wh, j, :, fc * 128:(fc + 1) * 128],
                        x_hl[:, j, :, xh, tg * 512:(tg + 1) * 512],
                        start=(i == 0), stop=(i == n - 1), perf_mode=DR,
                    )
                    i += 1
                    if i < n:
                        yield
            nc.vector.tensor_copy(dst[fc][:, tg * 512:(tg + 1) * 512], ps[:])

        def v_tile_gen(c):
            """V projection for key chunk c: out [128 tok, 512 feat]."""
            # 2-term variant keeps (x_hi + x_lo) @ wv_hi = x @ wv_hi exactly
            terms = VV_TERMS if c < VC3 else ((0, 0), (1, 0))
            ps = psW.tile([128, 512], F32, tag="w", name="v_ps")
            n = len(terms) * NJ
            i = 0
            for (xh, wh) in terms:
                for j in range(NJ):
                    nc.tensor.matmul(
                        ps[:],
                        x_hl[:, j, :, xh, c * 128:(c + 1) * 128],
                        w_sb["wv"][:, wh, j],
                        start=(i == 0), stop=(i == n - 1), perf_mode=DR,
                    )
                    i += 1
                    if i < n:
                        yield
            nc.vector.tensor_copy(
                va_sb[c][:, :, 0:64],
                ps[:].rearrange("p (h d) -> p h d", h=HL),
            )

        # o @ wo via fp8 DR: otn/wo hi+lo, 3 terms, K=256 per matmul
        O_TERMS = ((otn8h, wo8h), (otn8l, wo8h), (otn8h, wo8l))

        def o_group_gen(tb, half, alt=False):
            """Output projection for one [128 tok, 512 out] block."""
            if alt:  # tail: borrow an idle S-pair bank for deeper pipelining
                ps = psS.tile([128, 1024], F32, tag="s", name="f_ps")[:, 0:512]
            else:
                ps = psW.tile([128, 512], F32, tag="w", name="f_ps")
            i = 0
            for a in (0, 1):
                for (s_t, m_t) in O_TERMS:
                    nc.tensor.matmul(
                        ps[:],
                        s_t[:, a, :, tb, :],
                        m_t[:, a, :, half * 512:(half + 1) * 512],
                        start=(i == 0), stop=(i == 5), perf_mode=DR,
                    )
                    i += 1
                    if i < 6:
                        yield
            fin = fin_p.tile([128, 512], BF16, tag="fin", name="fin")
            # always DVE: keeps the scalar engine exclusively on exp
            nc.vector.tensor_scalar_mul(fin[:], ps[:], 1.0 / (WS * WS))
            nc.sync.dma_start(
                y_t[tb * 128:(tb + 1) * 128, half * 512:(half + 1) * 512], fin[:]
            )

        def run_gen(g):
            for _ in g:
                pass

        def kq_tile(w, dst, fc, tg):
            run_gen(kq_tile_gen(w, dst, fc, tg))

        def v_tile(c):
            run_gen(v_tile_gen(c))

        def o_group(tb, half):
            run_gen(o_group_gen(tb, half))

        # ---- pair order: staircase over (fc, qg) so the exp-heavy large
        # query groups start early and hide the front-loaded projection
        # filler, instead of running all qg0 pairs (PE-bound, tiny exp)
        # first and all qg3 pairs (Act-bound) last.
        PAIR_ORDER = TUNE.get("pair_order") or [
            (0, 0), (1, 0), (0, 1), (1, 1), (2, 0), (0, 2),
            (2, 1), (1, 2), (3, 0), (0, 3), (2, 2), (3, 1),
            (1, 3), (2, 3), (3, 2), (3, 3)]
        POS = {p: i for i, p in enumerate(PAIR_ORDER)}

        # ---- filler schedule: emit each tile as LATE as its deadline
        # allows.  Fillers carry a due-slot (= pair position); the attention
        # chunk loop pops fillers whose due-slot has been reached, weaving
        # them between exp and AV so they cover the scalar-engine latency.
        slots = [[] for _ in range(16)]

        def KQ(fc, tg):
            return [(lambda: kq_tile_gen("wk", kt_sb, fc, tg), ("kt", fc, tg)),
                    (lambda: kq_tile_gen("wq", qt_sb, fc, tg), ("qt", fc, tg))]

        def V(c):
            return [(lambda c=c: v_tile_gen(c), ("v", c))]

        def Og(tb, half):
            return [lambda: o_group_gen(tb, half)]

        # pair 1's kq first (its S/exp gate), then v0-3: pair (0,0)'s S/exp
        # emissions precede the pump calls, so the v generators can stall on
        # the wv DMA without blocking the exp chain.
        slots[0] += KQ(*PAIR_ORDER[1])
        slots[0] += V(0) + V(1) + V(2) + V(3)
        # kq(fc, tg) is used at position POS[(fc, tg)]; due ~2 earlier.
        for tg in range(4):
            for fc in range(4):
                if (fc, tg) in (PAIR_ORDER[0], PAIR_ORDER[1]):
                    continue  # pre-phase / already queued above
                slots[max(0, POS[(fc, tg)] - 2)] += KQ(fc, tg)
        # V(4tg+i) is needed once the first qg==tg pair reaches chunk 4tg+i.
        # Due POS-2: the overdue boundary drain then guarantees the va copy
        # is EMITTED before any AV that reads it (the dep tracker orders by
        # emission; a later-emitted copy would be serialized AFTER the AV
        # and the AV would read pre-copy garbage).
        for tg in (1, 2, 3):
            for i in range(4):
                slots[max(0, POS[(0, tg)] - 2)] += V(4 * tg + i)
        # fill_q entries: (due_slot, thunk, tag); tags let consumers
        # force-complete a producer's emission before the consuming
        # instruction is emitted (the dep tracker orders by emission).
        fill_q = []
        for s, ths in enumerate(slots):
            for th in ths:
                fill_q.append((s,) + (th if isinstance(th, tuple) else (th, None)))
        fill_q.reverse()  # pop from the end

        # output projections have no downstream consumer: keep them OUT of
        # the deadline queue so boundary drains never bulk-emit them while
        # the scalar engine idles; they are pulled opportunistically inside
        # the chunk loops, gated on ALL FOUR (fc, tg) hi/lo splits of their
        # token group having been EMITTED (else the dep tracker would order
        # the split after the o-group read).
        hilo_done = [0, 0, 0, 0]  # per-tg emit_hilo count
        opp_q = []
        for tg in (0, 1, 2):
            due = max(POS[(fc, tg)] for fc in range(4)) + 1
            due = min(15, due + TUNE["og_delay"])
            for tb in range(4 * tg, 4 * tg + 4):
                for half in (0, 1):
                    opp_q.append((due, tg, Og(tb, half)[0]))
        opp_q.sort(key=lambda e: e[0])
        opp_q.reverse()

        # ---- pre-phase: only what pair (fc0, qg0)'s S/exp chain needs.
        kq_tile("wk", kt_sb, 0, 0)
        kq_tile("wq", qt_sb, 0, 0)

        # ---- attention ---------------------------------------------------
        fill_state = {"gen": None, "vc": None}

        def pump(s_cur, chunks=1, opp=True, due=None):
            """Advance filler emission by ~`chunks` matmul groups."""
            d = s_cur if due is None else due
            while chunks > 0:
                if fill_state["gen"] is None:
                    if fill_q and fill_q[-1][0] <= d:
                        _, th, vc = fill_q.pop()
                        fill_state["gen"] = th()
                        fill_state["vc"] = vc
                    elif (opp and opp_q and opp_q[-1][0] <= d
                          and hilo_done[opp_q[-1][1]] >= 4):
                        fill_state["gen"] = opp_q.pop()[2]()
                        fill_state["vc"] = None
                    else:
                        return
                try:
                    next(fill_state["gen"])
                except StopIteration:
                    fill_state["gen"] = None
                    fill_state["vc"] = None
                chunks -= 1

        def ensure(tag):
            """Force-complete a tagged filler's emission (no-op if done)."""
            if fill_state["gen"] is not None and fill_state["vc"] == tag:
                for _ in fill_state["gen"]:
                    pass
                fill_state["gen"] = None
                fill_state["vc"] = None
                return
            for idx in range(len(fill_q) - 1, -1, -1):
                if fill_q[idx][2] == tag:
                    _, th, _ = fill_q.pop(idx)
                    for _ in th():
                        pass
                    return

        # transposes run on the DMA XBAR ([q,hd] -> [hd,q] per 128-block);
        # the fp8 hi/lo split for the DR output projection is deferred one
        # pair so the DVE ops never queue-stall on the in-flight DMA.
        pend_t = {"v": None}

        def emit_hilo(fc, qg):
            a, r = fc // 2, fc % 2
            sl = (slice(None), a, r, slice(qg * 4, (qg + 1) * 4), slice(None))
            nc.vector.tensor_copy(otn8h[sl], otnB[sl])
            nc.vector.tensor_tensor(otn8l[sl], otnB[sl], otn8h[sl], sub)
            hilo_done[qg] += 1

        def flush_transposes():
            if pend_t["v"] is None:
                return
            if len(pend_t["v"]) == 2:
                emit_hilo(*pend_t["v"])
            else:
                # last pair: PE transposes (PE is idle at the tail; the DMA
                # XBAR path would add ~2.7us to the critical tail chain)
                fc, qg, nt4 = pend_t["v"]
                a, r = fc // 2, fc % 2
                while fill_state["gen"] is not None:  # psW round-robin safety
                    try:
                        next(fill_state["gen"])
                    except StopIteration:
                        fill_state["gen"] = None
                for qb in range(4):
                    pt = psW.tile([128, 128], BF16, tag="w", name="pst")
                    nc.tensor.transpose(pt[:], nt4[:, qb, :], ident[:])
                    nc.vector.tensor_copy(otnB[:, a, r, qg * 4 + qb, :], pt[:])
                emit_hilo(fc, qg)
            pend_t["v"] = None

        def attention(fc, qg, carry, s_cur, defer_av=False):
            """Emit one head-pair's attention; returns a closure with the
            tail work (last AVs + normalize) that the NEXT head-pair runs
            after its first S/exp, keeping the scalar engine fed across
            boundaries.  defer_av pushes ALL AVs into the tail — used for
            pair (0,0), whose va tiles are only EMITTED during its chunk
            loop (an AV emitted before the va copy would be ordered ahead
            of it by the dep tracker and read garbage)."""
            nch = 4 * qg + 4
            o4c = [None, None]  # lazy per-head accumulators
            stash = []

            def get_o4(i):
                if o4c[i] is None:
                    o_ps = psO.tile([128, 260], F32, tag="o", name="o_ps")
                    o4c[i] = o_ps[:].rearrange("p (q x) -> p q x", q=4)
                return o4c[i]

            prev = None  # (p_tile, cc)
            ensure(("qt", fc, qg))  # qt tile emitted before any S reads

            def do_av(p_tile, cc):
                # all four 65-col regions share one PSUM bank: exactly one
                # start (zeroes the bank) and one stop per accumulator
                ensure(("v", cc))  # va copy must be EMITTED before this read
                M = max(0, cc - 4 * qg)
                for i in (0, 1):
                    o4 = get_o4(i)
                    for qb in range(M, 4):
                        nc.tensor.matmul(
                            o4[:, qb, :],
                            p_tile[:, i * 512 + qb * 128: i * 512 + (qb + 1) * 128],
                            va_sb[cc][:, 2 * fc + i, :],
                            start=(cc == 0 and qb == 0),
                            stop=(cc == nch - 1 and qb == 3),
                        )

            def emit_S(cc):
                ensure(("kt", fc, cc // 4))  # kt tile emitted before S reads
                s_ps = psS.tile([128, 1024], F32, tag="s", name="s_ps")
                p_tile = p_p.tile([128, 1024], BF16, tag="p", name="p_tile")
                M = max(0, cc - 4 * qg)
                for i in (0, 1):
                    nc.tensor.matmul(
                        s_ps[:, i * 512 + M * 128: (i + 1) * 512],
                        kt_sb[fc][i * 64:(i + 1) * 64, cc * 128:(cc + 1) * 128],
                        qt_sb[fc][i * 64:(i + 1) * 64,
                                  qg * 512 + M * 128:(qg + 1) * 512],
                        start=True, stop=True,
                    )
                return s_ps, p_tile

            cur = emit_S(0)
            for cc in range(nch):
                s_ps, p_tile = cur
                M = max(0, cc - 4 * qg)
                # one exp covers both heads (same trim)
                s_act = s_ps[:].rearrange("p (h x) -> p h x", h=2)[
                    :, :, M * 128: 512]
                p_act = p_tile[:].rearrange("p (h x) -> p h x", h=2)[
                    :, :, M * 128: 512]
                nc.scalar.activation(p_act, s_act, Exp, scale=1.0 / (8 * WS * WS))
                # keep the scalar engine fed: next chunk's S goes out
                # before filler and AV work
                if cc + 1 < nch:
                    cur = emit_S(cc + 1)
                # diagonal-block mask multiply, both heads in one op
                dqb = cc - 4 * qg
                if 0 <= dqb < 4:
                    psl = p_tile[:].rearrange("p (h x) -> p h x", h=2)[
                        :, :, dqb * 128:(dqb + 1) * 128]
                    # on GPSIMD: keeps the AV-feeding op off the DVE queue
                    nc.gpsimd.tensor_tensor(
                        psl, psl,
                        em_sb[:].unsqueeze(1).broadcast_to((128, 2, 128)),
                        mult,
                    )
                # run the previous pair's tail right after this pair's
                # first S/exp is in flight, then flush its transposes
                if cc == 0 and carry is not None:
                    carry()
                if cc == 3:
                    flush_transposes()
                # weave filler between exp and the dependent AVs so PE
                # covers the scalar-engine latency without starving it;
                # run much harder early (PE idles between tiny qg0 exps)
                # and when capped boundary drains left a backlog
                if TUNE["pump_early"] and s_cur <= 1:
                    n = TUNE["pump_early"]
                elif fill_q and fill_q[-1][0] < s_cur:
                    n = TUNE["pump_fast"]
                else:
                    n = TUNE["pump_base"]
                if cc + 1 == nch:
                    # last chunk: don't park filler ahead of the next
                    # pair's S/exp chain
                    n = min(n, TUNE["pump_last"])
                pump(s_cur, chunks=n)
                if prev is not None:
                    if defer_av:
                        stash.append(prev)
                    else:
                        do_av(*prev)
                prev = (p_tile, cc)

            last = prev

            def tail():
                for pc in stash:
                    do_av(*pc)
                do_av(*last)
                # normalize: rec of denominators (col 64 of each 65-group)
                nt4 = nt_p.tile([128, 4, 128], BF16, tag="nt", name="nt4")
                for i in (0, 1):
                    o4 = get_o4(i)
                    rec4 = rec_p.tile([128, 4], F32, tag="rec", name="rec4")
                    nc.vector.reciprocal(rec4[:].unsqueeze(-1), o4[:, :, 64:65])
                    nc.vector.tensor_tensor(
                        nt4[:, :, i * 64:(i + 1) * 64],
                        o4[:, :, 0:64],
                        rec4[:].unsqueeze(-1).broadcast_to((128, 4, 64)),
                        mult,
                    )
                if (fc, qg) == PAIR_ORDER[-1]:
                    pend_t["v"] = (fc, qg, nt4)  # PE-transposed at the tail
                else:
                    a, r = fc // 2, fc % 2
                    nc.sync.dma_start_transpose(
                        otnB[:, a, r, qg * 4:(qg + 1) * 4, :], nt4[:]
                    )
                    pend_t["v"] = (fc, qg)

            return tail

        carry = None
        for pos, (fc, qg) in enumerate(PAIR_ORDER):
            carry = attention(fc, qg, carry, pos)
            # small opportunistic drain only: hard correctness is enforced
            # by ensure() tags at each consumer, so nothing HAS to be
            # bulk-emitted between two pairs' S/exp chains.
            pump(pos, chunks=TUNE["drain_cap"])
        carry()
        pump(999, chunks=9999)
        flush_transposes()

        # tail: 8 output-projection groups for tb 12-15.  Phase A (fc0/fc1
        # DR terms) for six of them runs while the LAST pair's transpose +
        # hi/lo split is still in flight; phase B (fc2/fc3 terms) finishes
        # each group once otn8[a=1] lands.  Each accumulator is one full
        # PSUM bank (start zeroes the whole bank).
        def tail_A(ps, tb, half):
            for i, (s_t, m_t) in enumerate(O_TERMS):
                nc.tensor.matmul(
                    ps[:], s_t[:, 0, :, tb, :],
                    m_t[:, 0, :, half * 512:(half + 1) * 512],
                    start=(i == 0), stop=False, perf_mode=DR,
                )

        def tail_B(ps, tb, half, fin2):
            for i, (s_t, m_t) in enumerate(O_TERMS):
                nc.tensor.matmul(
                    ps[:], s_t[:, 1, :, tb, :],
                    m_t[:, 1, :, half * 512:(half + 1) * 512],
                    start=False, stop=(i == 2), perf_mode=DR,
                )
            dst = fin2[:, half * 512:(half + 1) * 512]
            if half:  # halves evacuated on Act and DVE in parallel
                nc.vector.tensor_scalar_mul(dst, ps[:], 1.0 / (WS * WS))
            else:
                nc.scalar.mul(dst, ps[:], 1.0 / (WS * WS))

        s0 = psS.tile([128, 1024], F32, tag="s", name="tailS0")
        s1 = psS.tile([128, 1024], F32, tag="s", name="tailS1")
        grp = [
            psW.tile([128, 512], F32, tag="w", name="tailW0"),
            psW.tile([128, 512], F32, tag="w", name="tailW1"),
            s0[:, 0:512], s0[:, 512:1024], s1[:, 0:512], s1[:, 512:1024],
            psO.tile([128, 512], F32, tag="o", name="tailO0"),
            psO.tile([128, 512], F32, tag="o", name="tailO1"),
        ]
        sched = [(12, 0), (12, 1), (13, 0), (13, 1),
                 (14, 0), (14, 1), (15, 0), (15, 1)]
        for ps, (tb, half) in zip(grp, sched):
            tail_A(ps, tb, half)
        fin2s = {}
        for ps, (tb, half) in zip(grp, sched):
            if tb not in fin2s:
                fin2s[tb] = fin_p.tile([128, 1024], BF16, tag="fin2", name="fin2")
            tail_B(ps, tb, half, fin2s[tb])
            if tb < 15 and half == 1:
                nc.sync.dma_start(y_t[tb * 128:(tb + 1) * 128, :], fin2s[tb][:])
            elif tb == 15:
                # final block: per-half DMAs so the last transfer chases its
                # own evacuation
                nc.sync.dma_start(
                    y_t[15 * 128:16 * 128, half * 512:(half + 1) * 512],
                    fin2s[15][:, half * 512:(half + 1) * 512])

        ctx.__exit__(None, None, None)

    nc.compile()
    return nc


# --------------------------------------------------------------------------
# host-side prep for the fast path
# --------------------------------------------------------------------------

def _q8(a):
    hi = a.astype(f8)
    lo = (a - hi.astype(np.float32)).astype(f8)
    return hi, lo


def _dr_layout(a):
    """[1024, N] contraction-major -> [512, 2N] DoubleRow layout.

    Row r = j*128 + p holds contraction indices d = 256j + 128i + p in
    column halves i = 0, 1 (per 128-row block).
    """
    n = a.shape[1]
    return np.ascontiguousarray(
        a.reshape(NJ, 2, 128, n).transpose(0, 2, 1, 3).reshape(512, 2 * n)
    )


def _dr_layout_x(a):
    """[1024, T] -> [128, NJ*2*T] partition-major DoubleRow layout."""
    n = a.shape[1]
    return np.ascontiguousarray(
        a.reshape(NJ, 2, 128, n).transpose(2, 0, 1, 3).reshape(128, NJ * 2 * n)
    )


def _dr_layout_xhl(xh, xl):
    """Two [1024, T] hi/lo planes -> [128, NJ*2*2*T] interleaved layout.

    x_hl[p, j, i, h, t] holds contraction index d = 256j + 128i + p of the
    h-th (hi/lo) plane.
    """
    n = xh.shape[1]
    planes = [a.reshape(NJ, 2, 128, n).transpose(2, 0, 1, 3) for a in (xh, xl)]
    return np.ascontiguousarray(
        np.stack(planes, axis=3).reshape(128, NJ * 2 * 2 * n)
    )


def _prep_fast_inputs(x, wq, wk, wv, wo):
    ins = []
    # p_tile is [key, query]: allowed iff key <= query -> upper triangular
    em = np.triu(np.ones((128, 128), dtype=np.float32)).astype(bf)
    xq = []
    for b in range(B):
        xh, xl = _q8(x[b].T)  # [1024, 2048]
        xq.append(_dr_layout_xhl(xh, xl))
    wqs = []
    for g in range(2):
        d = {}
        for name, w in (("wq", wq), ("wk", wk)):
            wt = np.ascontiguousarray(w[g * FG:(g + 1) * FG, :].T) * WS  # [1024, 512]
            hi, lo = _q8(wt)
            d[name + "8"] = np.stack([_dr_layout(hi), _dr_layout(lo)])
        wvt = np.ascontiguousarray(wv[g * FG:(g + 1) * FG, :].T) * WS
        wvh, wvl = _q8(wvt)
        d["wv8"] = np.stack(
            [_dr_layout(a) for a in ((wvh,) if V_TERMS == 2 else (wvh, wvl))]
        )
        # wo in fp8 DR layout [p, a, r, o]: feature f = 256a + 128r + p,
        # pre-scaled by WS so hi sits in e4m3's sweet spot (undone in fin).
        wot = np.ascontiguousarray(wo[:, g * FG:(g + 1) * FG].T) * WS  # [512, 1024]
        woh, wol = _q8(wot)
        d["wo8h"] = np.ascontiguousarray(
            woh.reshape(2, 2, 128, DIM).transpose(2, 0, 1, 3).reshape(128, -1)
        )
        d["wo8l"] = np.ascontiguousarray(
            wol.reshape(2, 2, 128, DIM).transpose(2, 0, 1, 3).reshape(128, -1)
        )
        wqs.append(d)
    for i in range(8):
        b, g = i // 2, i % 2
        im = {"xhl": xq[b], "em": em}
        im.update(wqs[g])
        ins.append(im)
    return ins


# --------------------------------------------------------------------------
# legacy general/nomask path (baseline kernel, kept as fallback)
# --------------------------------------------------------------------------

TQ = 1024
NDC = DIM // 128


def _causal_sched():
    sched = []
    for a in (0, 1):
        for c in range(8 * a + 8):
            sched.append((a, c, (c - 8 * a) // 2))
    return sched


CAUSAL_SCHED = _causal_sched()


def _build_legacy(variant, reps=1):
    """variant: 'general' | 'nomask' (original baseline kernel)."""
    nc = bacc.Bacc("TRN2", target_bir_lowering=False, debug=False, num_devices=8)

    xT = nc.dram_tensor("xT", [DIM, T], BF16, kind="ExternalInput").ap()
    xqT = nc.dram_tensor("xqT", [DIM, TQ], BF16, kind="ExternalInput").ap()
    wqT = nc.dram_tensor("wqT", [DIM, DIM], BF16, kind="ExternalInput").ap()
    wkT = nc.dram_tensor("wkT", [DIM, DIM], BF16, kind="ExternalInput").ap()
    wvT = nc.dram_tensor("wvT", [DIM, DIM], BF16, kind="ExternalInput").ap()
    woT = nc.dram_tensor("woT", [DIM, DIM], BF16, kind="ExternalInput").ap()
    if variant == "general":
        em = nc.dram_tensor("em", [T, TQ], BF16, kind="ExternalInput").ap()
    else:
        em = None
    out = nc.dram_tensor("out", [TQ, DIM], F32, kind="ExternalOutput").ap()

    with tile.TileContext(nc) as tc:
      for _rep in range(reps):
        ctx = ExitStack()
        ctx.__enter__()
        Exp = mybir.ActivationFunctionType.Exp
        mult = mybir.AluOpType.mult

        qt_p = ctx.enter_context(tc.tile_pool(name="qt", bufs=1))
        kt_p = ctx.enter_context(tc.tile_pool(name="kt", bufs=1))
        va_p = ctx.enter_context(tc.tile_pool(name="va", bufs=1))
        misc_p = ctx.enter_context(tc.tile_pool(name="misc", bufs=1))
        psS = ctx.enter_context(tc.tile_pool(name="psS", bufs=2, space="PSUM"))
        psO = ctx.enter_context(tc.tile_pool(name="psO", bufs=1, space="PSUM"))
        psB = ctx.enter_context(tc.tile_pool(name="psB", bufs=1, space="PSUM"))
        psF = ctx.enter_context(tc.tile_pool(name="psF", bufs=1, space="PSUM"))

        qt_sb = [qt_p.tile([128, TQ], BF16, tag=f"qt{i}", name=f"qt{i}") for i in range(NDC)]
        kt_sb = [kt_p.tile([128, T], BF16, tag=f"kt{i}", name=f"kt{i}") for i in range(NDC)]
        va_sb = [va_p.tile([128, H * 65], BF16, tag=f"va{i}", name=f"va{i}") for i in range(NTC)]
        ones_sb = misc_p.tile([128, 64], F32, tag="ones", name="ones")
        nc.vector.memset(ones_sb[64:65, :], 1.0)

        with tc.tile_pool(name="xin", bufs=1) as x_p, tc.tile_pool(
            name="win", bufs=1
        ) as w_p:
            xt_sb = [x_p.tile([128, T], BF16, tag=f"xt{i}", name=f"xt{i}") for i in range(NDC)]
            xq_sb = [x_p.tile([128, TQ], BF16, tag=f"xq{i}", name=f"xq{i}") for i in range(NDC)]
            wq_sb = [w_p.tile([128, DIM], BF16, tag=f"wq{i}", name=f"wq{i}") for i in range(NDC)]
            wk_sb = [w_p.tile([128, DIM], BF16, tag=f"wk{i}", name=f"wk{i}") for i in range(NDC)]
            wv_sb = [w_p.tile([128, DIM], BF16, tag=f"wv{i}", name=f"wv{i}") for i in range(NDC)]
            for i in range(NDC):
                s = slice(i * 128, (i + 1) * 128)
                nc.sync.dma_start(wv_sb[i][:], wvT[s, :])
                nc.sync.dma_start(xt_sb[i][:], xT[s, :])
            for i in range(NDC):
                s = slice(i * 128, (i + 1) * 128)
                nc.sync.dma_start(wq_sb[i][:], wqT[s, :])
                nc.sync.dma_start(xq_sb[i][:], xqT[s, :])
            for i in range(NDC):
                s = slice(i * 128, (i + 1) * 128)
                nc.sync.dma_start(wk_sb[i][:], wkT[s, :])

            for c in range(NTC):
                v4 = va_sb[c][:].rearrange("p (q t x) -> p q t x", q=8, t=2)
                nc.vector.memset(v4[:, :, 0:2, 64:65], 1.0)

            for c in range(NTC):
                csl = slice(c * 128, (c + 1) * 128)
                ps = psS.tile([128, 1024], F32, tag="s", name="v_ps")
                for n in range(2):
                    nsl = slice(n * 512, (n + 1) * 512)
                    for dc in range(NDC):
                        nc.tensor.matmul(
                            ps[:, nsl],
                            xt_sb[dc][:, csl],
                            wv_sb[dc][:, nsl],
                            start=(dc == 0),
                            stop=(dc == NDC - 1),
                        )
                v4 = va_sb[c][:].rearrange("p (q t x) -> p q t x", q=8, t=2)
                s4 = ps[:].rearrange("p (q t x) -> p q t x", q=8, t=2)
                nc.vector.tensor_copy(v4[:, :, 0:2, 0:64], s4[:, :, 0:2, :])

            for oc in range(NDC):
                osl = slice(oc * 128, (oc + 1) * 128)
                ps = psS.tile([128, 1024], F32, tag="s", name="q_ps")
                for n in range(2):
                    nsl = slice(n * 512, (n + 1) * 512)
                    for dc in range(NDC):
                        nc.tensor.matmul(
                            ps[:, nsl],
                            wq_sb[dc][:, osl],
                            xq_sb[dc][:, nsl],
                            start=(dc == 0),
                            stop=(dc == NDC - 1),
                        )
                nc.scalar.copy(qt_sb[oc][:], ps[:])
                for m in range(2):
                    ps = psS.tile([128, 1024], F32, tag="s", name="k_ps")
                    for n in range(2):
                        nsl = slice((2 * m + n) * 512, (2 * m + n + 1) * 512)
                        psl = slice(n * 512, (n + 1) * 512)
                        for dc in range(NDC):
                            nc.tensor.matmul(
                                ps[:, psl],
                                wk_sb[dc][:, osl],
                                xt_sb[dc][:, nsl],
                                start=(dc == 0),
                                stop=(dc == NDC - 1),
                            )
                    nc.scalar.copy(
                        kt_sb[oc][:, m * 1024: (m + 1) * 1024], ps[:]
                    )

        em_p = ctx.enter_context(tc.tile_pool(name="em", bufs=1))
        wo_p = ctx.enter_context(tc.tile_pool(name="wo", bufs=1))
        otn_p = ctx.enter_context(tc.tile_pool(name="otn", bufs=1))
        p_p = ctx.enter_context(tc.tile_pool(name="pp", bufs=6))
        osb_p = ctx.enter_context(tc.tile_pool(name="osb", bufs=2))
        nrm_p = ctx.enter_context(tc.tile_pool(name="nrm", bufs=2))
        fin_p = ctx.enter_context(tc.tile_pool(name="fin", bufs=2))

        if variant == "general":
            em_sb = [em_p.tile([128, TQ], BF16, tag=f"em{i}", name=f"emt{i}") for i in range(NTC)]
            for c in range(NTC):
                nc.sync.dma_start(em_sb[c][:], em[c * 128: (c + 1) * 128, :])
        wo_sb = [wo_p.tile([128, DIM], BF16, tag=f"wo{i}", name=f"wot{i}") for i in range(NDC)]
        for i in range(NDC):
            nc.sync.dma_start(wo_sb[i][:], woT[i * 128: (i + 1) * 128, :])
        otn_sb = [otn_p.tile([128, TQ], BF16, tag=f"otn{i}", name=f"otn{i}") for i in range(NDC)]
        otn1_sb = [otn_p.tile([64, TQ], BF16, tag=f"otn1{i}", name=f"otn1{i}") for i in range(NDC)]

        for a in (0, 1):
            nA = NTC
            asl = slice(a * 512, (a + 1) * 512)
            for hp in range(NDC):
                o_pair = psO.tile([128, 1024], F32, tag="opair", name="o_pair")
                for c in range(nA):
                    csl = slice(c * 128, (c + 1) * 128)
                    s_ps = psS.tile([128, 1024], F32, tag="s", name="s_ps")
                    nc.tensor.matmul(
                        s_ps[:, 0:512],
                        kt_sb[hp][0:64, csl],
                        qt_sb[hp][0:64, asl],
                        start=True,
                        stop=True,
                    )
                    nc.tensor.matmul(
                        s_ps[:, 512:1024],
                        kt_sb[hp][64:128, csl],
                        qt_sb[hp][64:128, asl],
                        start=True,
                        stop=True,
                        tile_position=(64, 0),
                    )
                    p_pair = p_p.tile([128, 1024], BF16, tag="p", name="p_pair")
                    nc.scalar.activation(p_pair[:], s_ps[:], Exp)
                    if variant == "general":
                        for half in (0, 1):
                            psl = slice(half * 512, half * 512 + 512)
                            nc.vector.tensor_mul(
                                p_pair[:, psl], p_pair[:, psl], em_sb[c][:, asl]
                            )
                    va4 = va_sb[c][:].rearrange("p (q t x) -> p q t x", q=8, t=2)
                    nc.tensor.matmul(
                        o_pair[0:65, 0:512],
                        va4[:, hp, 0, :],
                        p_pair[:, 0:512],
                        start=(c == 0),
                        stop=(c == nA - 1),
                    )
                    nc.tensor.matmul(
                        o_pair[0:65, 512:1024],
                        va4[:, hp, 1, :],
                        p_pair[:, 512:1024],
                        start=(c == 0),
                        stop=(c == nA - 1),
                    )
                o_sb = osb_p.tile([128, 1024], F32, tag="osb", name="o_sb")
                nc.vector.tensor_copy(o_sb[:], o_pair[:])
                rec = nrm_p.tile([128, 1024], mybir.dt.float32r, tag="rec", name="rec")
                with nc.allow_low_precision(reason="f32r recip for denom broadcast"):
                    nc.vector.reciprocal(rec[64:65, 0:1024], o_sb[64:65, 0:1024])
                b0 = psB.tile([64, 512], F32, tag="b", name="b0")
                b1 = psB.tile([64, 512], F32, tag="b", name="b1")
                nc.tensor.matmul(
                    b0[:],
                    ones_sb[64:65, :].bitcast(mybir.dt.float32r),
                    rec[64:65, 0:512],
                    start=True,
                    stop=True,
                    tile_position=(64, 0),
                )
                nc.tensor.matmul(
                    b1[:],
                    ones_sb[64:65, :].bitcast(mybir.dt.float32r),
                    rec[64:65, 512:1024],
                    start=True,
                    stop=True,
                    tile_position=(64, 0),
                )
                nc.vector.tensor_tensor(
                    otn_sb[hp][0:64, asl], o_sb[0:64, 0:512], b0[:], mult
                )
                nc.vector.tensor_tensor(
                    otn1_sb[hp][0:64, asl], o_sb[0:64, 512:1024], b1[:], mult
                )
                nc.sync.dma_start(otn_sb[hp][64:128, asl], otn1_sb[hp][0:64, asl])

            for tt in range(4 * a, 4 * a + 4):
                tsl = slice((tt % 4) * 128 + a * 512, (tt % 4) * 128 + a * 512 + 128)
                fin = fin_p.tile([128, 1024], F32, tag="fin", name="fin")
                for n in range(2):
                    nsl = slice(n * 512, (n + 1) * 512)
                    f_ps = psF.tile([128, 512], F32, tag="fwo", name="f_ps")
                    for hp in range(NDC):
                        nc.tensor.matmul(
                            f_ps[:],
                            otn_sb[hp][:, tsl],
                            wo_sb[hp][:, nsl],
                            start=(hp == 0),
                            stop=(hp == NDC - 1),
                        )
                    nc.vector.tensor_copy(fin[:, nsl], f_ps[:])
                nc.sync.dma_start(out[tt * 128: (tt + 1) * 128, :], fin[:])

        ctx.__exit__(None, None, None)

    nc.compile()
    return nc


_NC_CACHE = {}


def _get_nc(variant):
    if variant not in _NC_CACHE:
        if variant == "causal":
            _NC_CACHE[variant] = _build_fast()
        else:
            _NC_CACHE[variant] = _build_legacy(variant)
    return _NC_CACHE[variant]


def _kernel_legacy(x, emask, wq, wk, wv, wo, variant):
    scale = 1.0 / np.sqrt(HD)
    wqT = np.ascontiguousarray((wq * scale).T).astype(bf)
    wkT = np.ascontiguousarray(wk.T).astype(bf)
    wvT = np.ascontiguousarray(wv.T).astype(bf)
    woT = np.ascontiguousarray(wo.T).astype(bf)

    perms = {}
    for p in (0, 1):
        perms[p] = np.concatenate(
            [np.arange(128) + 128 * j for j in range(p, 16, 2)]
        )

    in_maps = []
    for i in range(8):
        b, p = i // 2, i % 2
        perm = perms[p]
        xb = x[b]
        im = {
            "xT": np.ascontiguousarray(xb.T).astype(bf),
            "xqT": np.ascontiguousarray(xb[perm].T).astype(bf),
            "wqT": wqT,
            "wkT": wkT,
            "wvT": wvT,
            "woT": woT,
        }
        if variant == "general":
            im["em"] = np.ascontiguousarray(emask.T[:, perm]).astype(bf)
        in_maps.append(im)

    nc = _get_nc(variant)
    res = run_bass_kernel_spmd(nc, in_maps, core_ids=list(range(8)))

    out_full = np.empty((B, T, DIM), dtype=np.float32)
    for i in range(8):
        b, p = i // 2, i % 2
        out_full[b, perms[p]] = res.results[i]["out"]
    return out_full


def kernel(x, mask, wq, wk, wv, wo):
    x = np.asarray(x, dtype=np.float32)
    mask = np.asarray(mask, dtype=np.float32)
    wq = np.asarray(wq, dtype=np.float32)
    wk = np.asarray(wk, dtype=np.float32)
    wv = np.asarray(wv, dtype=np.float32)
    wo = np.asarray(wo, dtype=np.float32)

    m2 = mask[0, 0]
    emask = np.exp(np.minimum(m2, 60.0)).astype(np.float32)
    tril = np.tril(np.ones((T, T), dtype=np.float32))
    if np.array_equal(emask, tril):
        variant = "causal"
    elif np.all(m2 == 0.0):
        variant = "nomask"
    else:
        variant = "general"

    if variant != "causal":
        return _kernel_legacy(x, emask, wq, wk, wv, wo, variant)

    in_maps = _prep_fast_inputs(x, wq, wk, wv, wo)
    nc = _get_nc("causal")
    res = run_bass_kernel_spmd(nc, in_maps, core_ids=list(range(8)))

    out_full = np.empty((B, T, DIM), dtype=np.float32)
    for b in range(B):
        y0 = res.results[2 * b]["y"].astype(np.float32)
        y1 = res.results[2 * b + 1]["y"].astype(np.float32)
        out_full[b] = y0 + y1
    return out_full



# revision 89
# speedup vs baseline: 1.0002x; 1.0002x over previous
"""Multi-head attention kernel for 8 Trainium2 NeuronCores.

Problem: B=4, T=2048, DIM=1024, 16 heads, head_dim=64, additive causal mask.
  q,k,v = x@W{q,k,v}.T ; attn = softmax(q k^T/8 + mask) ; out = (attn v)@Wo.T

Sharding (no collectives): core i handles batch i//2 and head-group i%2
(8 heads).  Each core projects q/k/v for its 8 heads only (512 features,
no duplicated projection work), runs full causal attention for those heads,
and computes a partial output projection (contraction over its 512
features).  The host sums the two partial outputs per batch while
unsharding.

On-chip math:
 - Q/K projections run as fp8(e4m3) DoubleRow matmuls (K=256 per instr,
   0.5 cyc/row) with a hi+lo 3-term split (x_hi*w_hi + x_hi*w_lo +
   x_lo*w_hi) for near-bf16 accuracy at 2x bf16 speed; x hi/lo planes
   are interleaved in one SBUF tile.  The V projection uses the 3-term
   split only for the first key chunk(s): later rows average many keys,
   so dropping x*wv_lo there is ~0.6% rms and invisible in max error.
 - Scores S[k,q] accumulate in PSUM fp32 (bf16 operands); exp via the
   scalar engine with scale=1/(8*32*32); causal handled by skipping
   fully-masked 128-blocks and one tril-mask multiply per diagonal
   block (on GPSIMD, keeping DVE free).
 - AV uses P as the stationary operand and [V|1] as the moving operand
   (65 cols), giving the softmax denominator for free in column 64.
   Attention output is normalized on DVE, block-transposed [q,hd] ->
   [hd,q] on the DMA XBAR (14ns/16x128 tile; the LAST pair uses PE
   transposes instead - the XBAR round trip would sit on the tail), and
   split into fp8 hi/lo for the output projection.
 - The output projection runs fp8 DR 3-term over (otn, wo) hi/lo pairs
   with feature pairs r-interleaved (K=256/instr): 25% cheaper than
   bf16.  All scales (x32 on wq/wk/wv/wo) fold into the exp scale and a
   1/1024 factor on the final PSUM evacuation.

Scheduling: per-(head-pair, query-group) chunk loops; pairs run in a
STAIRCASE order over (fc, qg) so the exp-heavy large query groups start
early and hide the front-loaded projection filler.  Each key chunk's
scores for both heads share one [128,1024] PSUM tile and ONE exp
instruction; the next chunk's S matmuls are emitted one chunk ahead of
filler and AV work so the scalar engine's feed never queues behind
them.  Projection tiles act as PE filler, emitted as generators at
single-matmul granularity and woven between exp and the dependent AVs
(due-slot hints; NO bulk boundary drains).  Correctness of emission
order - the tile dep-tracker orders by emission, so a consumer emitted
before its producer would read garbage - is enforced by ensure() tags:
S forces its kt/qt tiles, AV forces its va tile, o-groups gate on a
per-token-group hi/lo counter.  Each pair's tail (last AVs + normalize
+ transpose) is carried into the next pair after its first S/exp.  The
final four token blocks' output projections are split into phase A
(fc0/1 terms, runs during the last pair's transpose+hi/lo) and phase B
across all 8 PSUM banks.  PSUM: 2x[128,1024] S + 2x[128,260] per-head
accumulators + 2x[128,512] projection tiles = 8 banks; each
accumulator uses one start/stop per PSUM bank (hardware zeroes whole
banks).  DMAs are few and large (HWDGE issue is ~650ns serial), with
the startup chain split by hi/lo plane so kt0/qt0 begin ASAP.
"""

import sys
import numpy as np

sys.path.insert(0, "/opt/trn_rl_repo")

import ml_dtypes  # noqa: E402
from contextlib import ExitStack  # noqa: E402
from concourse import bass, bacc, tile  # noqa: E402
from concourse.bass_utils import run_bass_kernel_spmd  # noqa: E402
from concourse.masks import make_identity  # noqa: E402

mybir = bass.mybir

B, T, DIM, H, HD = 4, 2048, 1024, 16, 64
HL = 8             # heads per core (head-group)
FG = 512           # features per core (HL * HD)
NJ = 4             # DoubleRow contraction chunks (256 each)
NTC = T // 128     # 16 key chunks
BF16 = mybir.dt.bfloat16
F8 = mybir.dt.float8e4
F32 = mybir.dt.float32
WS = 32.0          # weight pre-scale for e4m3
f8 = ml_dtypes.float8_e4m3fn
bf = ml_dtypes.bfloat16


# --------------------------------------------------------------------------
# fast causal path
# --------------------------------------------------------------------------

V_TERMS = 3       # 2: v = x @ wv_hi (drops the wv_lo correction); 3: full

# schedule tuning knobs (overridden by the sweep harness)
TUNE = {
    "pump_early": 12,   # in-loop pump chunks for positions <= 1 (0: off)
    "pump_fast": 10,    # in-loop pump chunks when a drain backlog exists
    "pump_base": 3,     # in-loop pump chunks otherwise
    "drain_cap": 4,     # boundary drain step cap
    "pump_last": 99,    # pump cap on each pair's final chunk
    "pair_order": None,  # override (fc, qg) processing order
    "vc3": 1,           # key chunks with full 3-term V projection
    "warmup": 0,        # dummy PE matmuls at t=0 (p-state ramp resets on
                        # idle in the cost model, so this is off)
    "og_delay": 0,      # extra positions to defer o-group fillers by
}


def _build_fast(reps=1):
    nc = bacc.Bacc("TRN2", target_bir_lowering=False, debug=False, num_devices=8)
    DR = mybir.MatmulPerfMode.DoubleRow
    Exp = mybir.ActivationFunctionType.Exp
    mult = mybir.AluOpType.mult
    sub = mybir.AluOpType.subtract

    # x hi/lo interleaved: [p, j, i, h, t]; contraction d = 256j + 128i + p
    xhl_t = nc.dram_tensor("xhl", [128, NJ * 2 * 2 * T], F8, kind="ExternalInput").ap()
    # wq/wk hi+lo merged per weight: [h, (j p), (i f)]
    w_t = {}
    for w in ("wq", "wk"):
        w_t[w] = nc.dram_tensor(w + "8", [2, 512, 1024], F8, kind="ExternalInput").ap()
    w_t["wv"] = nc.dram_tensor(
        "wv8", [V_TERMS - 1, 512, 1024], F8, kind="ExternalInput"
    ).ap()
    # wo hi/lo in DR layout: [p, a, r, o]; feature f = 256a + 128r + p
    wo8h_t = nc.dram_tensor("wo8h", [128, 2 * 2 * DIM], F8, kind="ExternalInput").ap()
    wo8l_t = nc.dram_tensor("wo8l", [128, 2 * 2 * DIM], F8, kind="ExternalInput").ap()
    em_t = nc.dram_tensor("em", [128, 128], BF16, kind="ExternalInput").ap()
    y_t = nc.dram_tensor("y", [T, DIM], BF16, kind="ExternalOutput").ap()

    with tile.TileContext(nc) as tc:
      for _rep in range(reps):
        ctx = ExitStack()
        ctx.__enter__()

        # ---- pools -------------------------------------------------------
        x_p = ctx.enter_context(tc.tile_pool(name="xp", bufs=1))
        w_p = ctx.enter_context(tc.tile_pool(name="wp", bufs=1))
        wo_p = ctx.enter_context(tc.tile_pool(name="wop", bufs=1))
        kt_p = ctx.enter_context(tc.tile_pool(name="ktp", bufs=1))
        qt_p = ctx.enter_context(tc.tile_pool(name="qtp", bufs=1))
        va_p = ctx.enter_context(tc.tile_pool(name="vap", bufs=1))
        misc_p = ctx.enter_context(tc.tile_pool(name="miscp", bufs=1))
        otn_p = ctx.enter_context(tc.tile_pool(name="otnp", bufs=1))
        p_p = ctx.enter_context(tc.tile_pool(name="pp", bufs=6))
        nt_p = ctx.enter_context(tc.tile_pool(name="ntp", bufs=3))
        rec_p = ctx.enter_context(tc.tile_pool(name="recp", bufs=4))
        fin_p = ctx.enter_context(tc.tile_pool(name="finp", bufs=4))
        # PSUM: s 2x[128,1024] = 4 banks, o 2x[128,260] = 2 (shared with the
        # [128,128] transpose outputs), w 2x[128,512] = 2  -> 8 banks.
        psS = ctx.enter_context(tc.tile_pool(name="psS", bufs=2, space="PSUM"))
        psO = ctx.enter_context(tc.tile_pool(name="psO", bufs=2, space="PSUM"))
        psW = ctx.enter_context(tc.tile_pool(name="psW", bufs=2, space="PSUM"))

        # ---- persistent SBUF tiles --------------------------------------
        x_hl = x_p.tile([128, NJ, 2, 2, T], F8, tag="xhl", name="x_hl")
        w_sb = {}
        for w in ("wq", "wk"):
            w_sb[w] = w_p.tile([128, 2, NJ, 2, FG], F8, tag=w, name=w)
        w_sb["wv"] = w_p.tile(
            [128, V_TERMS - 1, NJ, 2, FG], F8, tag="wv", name="wv"
        )
        # wo / otn fp8 DR tiles: [p, a, r, ...]; feature f = 256a + 128r + p
        wo8h = wo_p.tile([128, 2, 2, DIM], F8, tag="wo8h", name="wo8h")
        wo8l = wo_p.tile([128, 2, 2, DIM], F8, tag="wo8l", name="wo8l")
        kt_sb = [kt_p.tile([128, T], BF16, tag=f"kt{i}", name=f"kt{i}") for i in range(4)]
        qt_sb = [qt_p.tile([128, T], BF16, tag=f"qt{i}", name=f"qt{i}") for i in range(4)]
        va_sb = [va_p.tile([128, HL, 65], BF16, tag=f"va{i}", name=f"va{i}") for i in range(NTC)]
        otnB = otn_p.tile([128, 2, 2, 16, 128], BF16, tag="otnB", name="otnB")
        otn8h = otn_p.tile([128, 2, 2, 16, 128], F8, tag="otn8h", name="otn8h")
        otn8l = otn_p.tile([128, 2, 2, 16, 128], F8, tag="otn8l", name="otn8l")
        em_sb = misc_p.tile([128, 128], BF16, tag="em", name="em_sb")
        ident = misc_p.tile([128, 128], BF16, tag="id", name="ident")
        make_identity(nc, ident[:])

        # PE warm-up: the tensor engine runs at 1/2-1/4 clock until it has
        # been continuously busy ~3us.  Burn that ramp on dummy matmuls
        # over scratch data while the first input DMAs are still in
        # flight, so the real projections run at full clock.
        if TUNE["warmup"]:
            wm_sb = misc_p.tile([128, 512], BF16, tag="wm", name="wm_sb")
            nc.vector.memset(wm_sb[:], 1.0)
            wm_ps = psW.tile([128, 512], F32, tag="w", name="wm_ps")
            n = TUNE["warmup"]
            for i in range(n):
                nc.tensor.matmul(
                    wm_ps[:], ident[:], wm_sb[:],
                    start=(i == 0), stop=(i == n - 1),
                )
            nc.vector.tensor_copy(wm_sb[:], wm_ps[:])

        # ---- input DMAs --------------------------------------------------
        # HWDGE serializes DMA issue (~650ns each) and transfers serialize on
        # the DMA engines, so strict first-needed order; the startup chain
        # (wk/x/wq for kt0+qt0) is split by hi/lo plane to unblock the first
        # projection matmuls as early as possible.
        def dma_w(key, h):
            nc.sync.dma_start(
                w_sb[key][:, h].rearrange("p a b c -> p a (b c)"),
                w_t[key][h].rearrange("(j p) f -> p j f", j=NJ),
            )

        def dma_wv():
            nc.sync.dma_start(
                w_sb["wv"][:].rearrange("p h a b c -> p h a (b c)"),
                w_t["wv"].rearrange("h (j p) f -> p h j f", j=NJ),
            )

        def dma_x(lo, hi, h=None):
            src = xhl_t[:].rearrange("p (j i h t) -> p j i h t", j=NJ, i=2, h=2)
            if h is None:
                nc.sync.dma_start(x_hl[:, :, :, :, lo:hi], src[:, :, :, :, lo:hi])
            else:
                nc.sync.dma_start(
                    x_hl[:, :, :, h, lo:hi], src[:, :, :, h, lo:hi]
                )

        dma_w("wk", 0)
        dma_x(0, 512, 0)
        dma_w("wk", 1)
        dma_x(0, 512, 1)
        dma_w("wq", 0)
        dma_w("wq", 1)
        dma_wv()
        nc.sync.dma_start(em_sb[:], em_t[:])
        dma_x(512, 1024)
        nc.sync.dma_start(
            wo8h[:], wo8h_t[:].rearrange("p (a r o) -> p a r o", a=2, r=2)
        )
        nc.sync.dma_start(
            wo8l[:], wo8l_t[:].rearrange("p (a r o) -> p a r o", a=2, r=2)
        )
        dma_x(1024, 1536)
        dma_x(1536, 2048)

        for c in range(NTC):
            nc.vector.memset(va_sb[c][:, :, 64:65], 1.0)

        # ---- projection tile emitters (generators yielding every matmul
        # so filler can be woven at fine granularity) ----------------------
        # term order (x h-idx, w h-idx): x-lo last, so the startup tiles can
        # begin before the x-lo plane's DMA lands
        KQ_TERMS = ((0, 0), (0, 1), (1, 0))
        VV_TERMS = KQ_TERMS if V_TERMS == 3 else ((0, 0), (1, 0))
        # early key chunks keep the full 3-term V projection: early-row
        # attention averages few keys, so the wv_lo correction matters
        # there; later rows average it away (error ~3.6%/sqrt(n_eff)).
        VC3 = TUNE.get("vc3", 2)

        def kq_tile_gen(w, dst, fc, tg):
            """K or Q projection tile: out [128 feat, 512 tok]."""
            ps = psW.tile([128, 512], F32, tag="w", name=f"{w}_ps")
            n = 3 * NJ
            i = 0
            for (xh, wh) in KQ_TERMS:
                for j in range(NJ):
                    nc.tensor.matmul(
                        ps[:],
                        w_sb[w][:, wh, j, :, fc * 128:(fc + 1) * 128],
                        x_hl[:, j, :, xh, tg * 512:(tg + 1) * 512],
                        start=(i == 0), stop=(i == n - 1), perf_mode=DR,
                    )
                    i += 1
                    if i < n:
                        yield
            nc.vector.tensor_copy(dst[fc][:, tg * 512:(tg + 1) * 512], ps[:])

        def v_tile_gen(c):
            """V projection for key chunk c: out [128 tok, 512 feat]."""
            # 2-term variant keeps (x_hi + x_lo) @ wv_hi = x @ wv_hi exactly
            terms = VV_TERMS if c < VC3 else ((0, 0), (1, 0))
            ps = psW.tile([128, 512], F32, tag="w", name="v_ps")
            n = len(terms) * NJ
            i = 0
            for (xh, wh) in terms:
                for j in range(NJ):
                    nc.tensor.matmul(
                        ps[:],
                        x_hl[:, j, :, xh, c * 128:(c + 1) * 128],
                        w_sb["wv"][:, wh, j],
                        start=(i == 0), stop=(i == n - 1), perf_mode=DR,
                    )
                    i += 1
                    if i < n:
                        yield
            nc.vector.tensor_copy(
                va_sb[c][:, :, 0:64],
                ps[:].rearrange("p (h d) -> p h d", h=HL),
            )

        # o @ wo via fp8 DR: otn/wo hi+lo, 3 terms, K=256 per matmul
        O_TERMS = ((otn8h, wo8h), (otn8l, wo8h), (otn8h, wo8l))

        def o_group_gen(tb, half, alt=False):
            """Output projection for one [128 tok, 512 out] block."""
            if alt:  # tail: borrow an idle S-pair bank for deeper pipelining
                ps = psS.tile([128, 1024], F32, tag="s", name="f_ps")[:, 0:512]
            else:
                ps = psW.tile([128, 512], F32, tag="w", name="f_ps")
            i = 0
            for a in (0, 1):
                for (s_t, m_t) in O_TERMS:
                    nc.tensor.matmul(
                        ps[:],
                        s_t[:, a, :, tb, :],
                        m_t[:, a, :, half * 512:(half + 1) * 512],
                        start=(i == 0), stop=(i == 5), perf_mode=DR,
                    )
                    i += 1
                    if i < 6:
                        yield
            fin = fin_p.tile([128, 512], BF16, tag="fin", name="fin")
            # always DVE: keeps the scalar engine exclusively on exp
            nc.vector.tensor_scalar_mul(fin[:], ps[:], 1.0 / (WS * WS))
            nc.sync.dma_start(
                y_t[tb * 128:(tb + 1) * 128, half * 512:(half + 1) * 512], fin[:]
            )

        def run_gen(g):
            for _ in g:
                pass

        def kq_tile(w, dst, fc, tg):
            run_gen(kq_tile_gen(w, dst, fc, tg))

        def v_tile(c):
            run_gen(v_tile_gen(c))

        def o_group(tb, half):
            run_gen(o_group_gen(tb, half))

        # ---- pair order: staircase over (fc, qg) so the exp-heavy large
        # query groups start early and hide the front-loaded projection
        # filler, instead of running all qg0 pairs (PE-bound, tiny exp)
        # first and all qg3 pairs (Act-bound) last.
        PAIR_ORDER = TUNE.get("pair_order") or [
            (0, 0), (1, 0), (0, 1), (1, 1), (2, 0), (0, 2),
            (2, 1), (1, 2), (3, 0), (0, 3), (2, 2), (3, 1),
            (1, 3), (2, 3), (3, 2), (3, 3)]
        POS = {p: i for i, p in enumerate(PAIR_ORDER)}

        # ---- filler schedule: emit each tile as LATE as its deadline
        # allows.  Fillers carry a due-slot (= pair position); the attention
        # chunk loop pops fillers whose due-slot has been reached, weaving
        # them between exp and AV so they cover the scalar-engine latency.
        slots = [[] for _ in range(16)]

        def KQ(fc, tg):
            return [(lambda: kq_tile_gen("wk", kt_sb, fc, tg), ("kt", fc, tg)),
                    (lambda: kq_tile_gen("wq", qt_sb, fc, tg), ("qt", fc, tg))]

        def V(c):
            return [(lambda c=c: v_tile_gen(c), ("v", c))]

        def Og(tb, half):
            return [lambda: o_group_gen(tb, half)]

        # pair 1's kq first (its S/exp gate), then v0-3: pair (0,0)'s S/exp
        # emissions precede the pump calls, so the v generators can stall on
        # the wv DMA without blocking the exp chain.
        slots[0] += KQ(*PAIR_ORDER[1])
        slots[0] += V(0) + V(1) + V(2) + V(3)
        # kq(fc, tg) is used at position POS[(fc, tg)]; due ~2 earlier.
        for tg in range(4):
            for fc in range(4):
                if (fc, tg) in (PAIR_ORDER[0], PAIR_ORDER[1]):
                    continue  # pre-phase / already queued above
                slots[max(0, POS[(fc, tg)] - 2)] += KQ(fc, tg)
        # V(4tg+i) is needed once the first qg==tg pair reaches chunk 4tg+i.
        # Due POS-2: the overdue boundary drain then guarantees the va copy
        # is EMITTED before any AV that reads it (the dep tracker orders by
        # emission; a later-emitted copy would be serialized AFTER the AV
        # and the AV would read pre-copy garbage).
        for tg in (1, 2, 3):
            for i in range(4):
                slots[max(0, POS[(0, tg)] - 2)] += V(4 * tg + i)
        # fill_q entries: (due_slot, thunk, tag); tags let consumers
        # force-complete a producer's emission before the consuming
        # instruction is emitted (the dep tracker orders by emission).
        fill_q = []
        for s, ths in enumerate(slots):
            for th in ths:
                fill_q.append((s,) + (th if isinstance(th, tuple) else (th, None)))
        fill_q.reverse()  # pop from the end

        # output projections have no downstream consumer: keep them OUT of
        # the deadline queue so boundary drains never bulk-emit them while
        # the scalar engine idles; they are pulled opportunistically inside
        # the chunk loops, gated on ALL FOUR (fc, tg) hi/lo splits of their
        # token group having been EMITTED (else the dep tracker would order
        # the split after the o-group read).
        hilo_done = [0, 0, 0, 0]  # per-tg emit_hilo count
        opp_q = []
        for tg in (0, 1, 2):
            due = max(POS[(fc, tg)] for fc in range(4)) + 1
            due = min(15, due + TUNE["og_delay"])
            for tb in range(4 * tg, 4 * tg + 4):
                for half in (0, 1):
                    opp_q.append((due, tg, Og(tb, half)[0]))
        opp_q.sort(key=lambda e: e[0])
        opp_q.reverse()

        # ---- pre-phase: only what pair (fc0, qg0)'s S/exp chain needs.
        kq_tile("wk", kt_sb, 0, 0)
        kq_tile("wq", qt_sb, 0, 0)

        # ---- attention ---------------------------------------------------
        fill_state = {"gen": None, "vc": None}

        def pump(s_cur, chunks=1, opp=True, due=None):
            """Advance filler emission by ~`chunks` matmul groups."""
            d = s_cur if due is None else due
            while chunks > 0:
                if fill_state["gen"] is None:
                    if fill_q and fill_q[-1][0] <= d:
                        _, th, vc = fill_q.pop()
                        fill_state["gen"] = th()
                        fill_state["vc"] = vc
                    elif (opp and opp_q and opp_q[-1][0] <= d
                          and hilo_done[opp_q[-1][1]] >= 4):
                        fill_state["gen"] = opp_q.pop()[2]()
                        fill_state["vc"] = None
                    else:
                        return
                try:
                    next(fill_state["gen"])
                except StopIteration:
                    fill_state["gen"] = None
                    fill_state["vc"] = None
                chunks -= 1

        def ensure(tag):
            """Force-complete a tagged filler's emission (no-op if done)."""
            if fill_state["gen"] is not None and fill_state["vc"] == tag:
                for _ in fill_state["gen"]:
                    pass
                fill_state["gen"] = None
                fill_state["vc"] = None
                return
            for idx in range(len(fill_q) - 1, -1, -1):
                if fill_q[idx][2] == tag:
                    _, th, _ = fill_q.pop(idx)
                    for _ in th():
                        pass
                    return

        # transposes run on the DMA XBAR ([q,hd] -> [hd,q] per 128-block);
        # the fp8 hi/lo split for the DR output projection is deferred one
        # pair so the DVE ops never queue-stall on the in-flight DMA.
        pend_t = {"v": None}

        def emit_hilo(fc, qg):
            a, r = fc // 2, fc % 2
            sl = (slice(None), a, r, slice(qg * 4, (qg + 1) * 4), slice(None))
            nc.vector.tensor_copy(otn8h[sl], otnB[sl])
            nc.vector.tensor_tensor(otn8l[sl], otnB[sl], otn8h[sl], sub)
            hilo_done[qg] += 1

        def flush_transposes():
            if pend_t["v"] is None:
                return
            if len(pend_t["v"]) == 2:
                emit_hilo(*pend_t["v"])
            else:
                # last pair: PE transposes (PE is idle at the tail; the DMA
                # XBAR path would add ~2.7us to the critical tail chain)
                fc, qg, nt4 = pend_t["v"]
                a, r = fc // 2, fc % 2
                while fill_state["gen"] is not None:  # psW round-robin safety
                    try:
                        next(fill_state["gen"])
                    except StopIteration:
                        fill_state["gen"] = None
                for qb in range(4):
                    pt = psW.tile([128, 128], BF16, tag="w", name="pst")
                    nc.tensor.transpose(pt[:], nt4[:, qb, :], ident[:])
                    nc.vector.tensor_copy(otnB[:, a, r, qg * 4 + qb, :], pt[:])
                emit_hilo(fc, qg)
            pend_t["v"] = None

        def attention(fc, qg, carry, s_cur, defer_av=False):
            """Emit one head-pair's attention; returns a closure with the
            tail work (last AVs + normalize) that the NEXT head-pair runs
            after its first S/exp, keeping the scalar engine fed across
            boundaries.  defer_av pushes ALL AVs into the tail — used for
            pair (0,0), whose va tiles are only EMITTED during its chunk
            loop (an AV emitted before the va copy would be ordered ahead
            of it by the dep tracker and read garbage)."""
            nch = 4 * qg + 4
            o4c = [None, None]  # lazy per-head accumulators
            stash = []

            def get_o4(i):
                if o4c[i] is None:
                    o_ps = psO.tile([128, 260], F32, tag="o", name="o_ps")
                    o4c[i] = o_ps[:].rearrange("p (q x) -> p q x", q=4)
                return o4c[i]

            prev = None  # (p_tile, cc)
            ensure(("qt", fc, qg))  # qt tile emitted before any S reads

            def do_av(p_tile, cc):
                # all four 65-col regions share one PSUM bank: exactly one
                # start (zeroes the bank) and one stop per accumulator
                ensure(("v", cc))  # va copy must be EMITTED before this read
                M = max(0, cc - 4 * qg)
                for i in (0, 1):
                    o4 = get_o4(i)
                    for qb in range(M, 4):
                        nc.tensor.matmul(
                            o4[:, qb, :],
                            p_tile[:, i * 512 + qb * 128: i * 512 + (qb + 1) * 128],
                            va_sb[cc][:, 2 * fc + i, :],
                            start=(cc == 0 and qb == 0),
                            stop=(cc == nch - 1 and qb == 3),
                        )

            def emit_S(cc):
                ensure(("kt", fc, cc // 4))  # kt tile emitted before S reads
                s_ps = psS.tile([128, 1024], F32, tag="s", name="s_ps")
                p_tile = p_p.tile([128, 1024], BF16, tag="p", name="p_tile")
                M = max(0, cc - 4 * qg)
                for i in (0, 1):
                    nc.tensor.matmul(
                        s_ps[:, i * 512 + M * 128: (i + 1) * 512],
                        kt_sb[fc][i * 64:(i + 1) * 64, cc * 128:(cc + 1) * 128],
                        qt_sb[fc][i * 64:(i + 1) * 64,
                                  qg * 512 + M * 128:(qg + 1) * 512],
                        start=True, stop=True,
                    )
                return s_ps, p_tile

            cur = emit_S(0)
            for cc in range(nch):
                s_ps, p_tile = cur
                M = max(0, cc - 4 * qg)
                # one exp covers both heads (same trim)
                s_act = s_ps[:].rearrange("p (h x) -> p h x", h=2)[
                    :, :, M * 128: 512]
                p_act = p_tile[:].rearrange("p (h x) -> p h x", h=2)[
                    :, :, M * 128: 512]
                nc.scalar.activation(p_act, s_act, Exp, scale=1.0 / (8 * WS * WS))
                # keep the scalar engine fed: next chunk's S goes out
                # before filler and AV work
                if cc + 1 < nch:
                    cur = emit_S(cc + 1)
                # diagonal-block mask multiply, both heads in one op
                dqb = cc - 4 * qg
                if 0 <= dqb < 4:
                    psl = p_tile[:].rearrange("p (h x) -> p h x", h=2)[
                        :, :, dqb * 128:(dqb + 1) * 128]
                    # on GPSIMD: keeps the AV-feeding op off the DVE queue
                    nc.gpsimd.tensor_tensor(
                        psl, psl,
                        em_sb[:].unsqueeze(1).broadcast_to((128, 2, 128)),
                        mult,
                    )
                # run the previous pair's tail right after this pair's
                # first S/exp is in flight, then flush its transposes
                if cc == 0 and carry is not None:
                    carry()
                if cc == 3:
                    flush_transposes()
                # weave filler between exp and the dependent AVs so PE
                # covers the scalar-engine latency without starving it;
                # run much harder early (PE idles between tiny qg0 exps)
                # and when capped boundary drains left a backlog
                if TUNE["pump_early"] and s_cur <= 1:
                    n = TUNE["pump_early"]
                elif fill_q and fill_q[-1][0] < s_cur:
                    n = TUNE["pump_fast"]
                else:
                    n = TUNE["pump_base"]
                if cc + 1 == nch:
                    # last chunk: don't park filler ahead of the next
                    # pair's S/exp chain
                    n = min(n, TUNE["pump_last"])
                pump(s_cur, chunks=n)
                if prev is not None:
                    if defer_av:
                        stash.append(prev)
                    else:
                        do_av(*prev)
                prev = (p_tile, cc)

            last = prev

            def tail():
                for pc in stash:
                    do_av(*pc)
                do_av(*last)
                # normalize: rec of denominators (col 64 of each 65-group)
                nt4 = nt_p.tile([128, 4, 128], BF16, tag="nt", name="nt4")
                for i in (0, 1):
                    o4 = get_o4(i)
                    rec4 = rec_p.tile([128, 4], F32, tag="rec", name="rec4")
                    nc.vector.reciprocal(rec4[:].unsqueeze(-1), o4[:, :, 64:65])
                    nc.vector.tensor_tensor(
                        nt4[:, :, i * 64:(i + 1) * 64],
                        o4[:, :, 0:64],
                        rec4[:].unsqueeze(-1).broadcast_to((128, 4, 64)),
                        mult,
                    )
                if (fc, qg) == PAIR_ORDER[-1]:
                    pend_t["v"] = (fc, qg, nt4)  # PE-transposed at the tail
                else:
                    a, r = fc // 2, fc % 2
                    nc.sync.dma_start_transpose(
                        otnB[:, a, r, qg * 4:(qg + 1) * 4, :], nt4[:]
                    )
                    pend_t["v"] = (fc, qg)

            return tail

        carry = None
        for pos, (fc, qg) in enumerate(PAIR_ORDER):
            carry = attention(fc, qg, carry, pos)
            # small opportunistic drain only: hard correctness is enforced
            # by ensure() tags at each consumer, so nothing HAS to be
            # bulk-emitted between two pairs' S/exp chains.
            pump(pos, chunks=TUNE["drain_cap"])
        carry()
        pump(999, chunks=9999)
        flush_transposes()

        # tail: 8 output-projection groups for tb 12-15.  Phase A (fc0/fc1
        # DR terms) for six of them runs while the LAST pair's transpose +
        # hi/lo split is still in flight; phase B (fc2/fc3 terms) finishes
        # each group once otn8[a=1] lands.  Each accumulator is one full
        # PSUM bank (start zeroes the whole bank).
        def tail_A(ps, tb, half):
            for i, (s_t, m_t) in enumerate(O_TERMS):
                nc.tensor.matmul(
                    ps[:], s_t[:, 0, :, tb, :],
                    m_t[:, 0, :, half * 512:(half + 1) * 512],
                    start=(i == 0), stop=False, perf_mode=DR,
                )

        def tail_B(ps, tb, half, fin2):
            for i, (s_t, m_t) in enumerate(O_TERMS):
                nc.tensor.matmul(
                    ps[:], s_t[:, 1, :, tb, :],
                    m_t[:, 1, :, half * 512:(half + 1) * 512],
                    start=False, stop=(i == 2), perf_mode=DR,
                )
            dst = fin2[:, half * 512:(half + 1) * 512]
            if half:  # halves evacuated on Act and DVE in parallel
                nc.vector.tensor_scalar_mul(dst, ps[:], 1.0 / (WS * WS))
            else:
                nc.scalar.mul(dst, ps[:], 1.0 / (WS * WS))

        s0 = psS.tile([128, 1024], F32, tag="s", name="tailS0")
        s1 = psS.tile([128, 1024], F32, tag="s", name="tailS1")
        grp = [
            psW.tile([128, 512], F32, tag="w", name="tailW0"),
            psW.tile([128, 512], F32, tag="w", name="tailW1"),
            s0[:, 0:512], s0[:, 512:1024], s1[:, 0:512], s1[:, 512:1024],
            psO.tile([128, 512], F32, tag="o", name="tailO0"),
            psO.tile([128, 512], F32, tag="o", name="tailO1"),
        ]
        sched = [(12, 0), (12, 1), (13, 0), (13, 1),
                 (14, 0), (14, 1), (15, 0), (15, 1)]
        for ps, (tb, half) in zip(grp, sched):
            tail_A(ps, tb, half)
        fin2s = {}
        for ps, (tb, half) in zip(grp, sched):
            if tb not in fin2s:
                fin2s[tb] = fin_p.tile([128, 1024], BF16, tag="fin2", name="fin2")
            tail_B(ps, tb, half, fin2s[tb])
            if tb < 15 and half == 1:
                nc.sync.dma_start(y_t[tb * 128:(tb + 1) * 128, :], fin2s[tb][:])
            elif tb == 15:
                # final block: per-half DMAs so the last transfer chases its
                # own evacuation
                nc.sync.dma_start(
                    y_t[15 * 128:16 * 128, half * 512:(half + 1) * 512],
                    fin2s[15][:, half * 512:(half + 1) * 512])

        ctx.__exit__(None, None, None)

    nc.compile()
    return nc


# --------------------------------------------------------------------------
# host-side prep for the fast path
# --------------------------------------------------------------------------

def _q8(a):
    hi = a.astype(f8)
    lo = (a - hi.astype(np.float32)).astype(f8)
    return hi, lo


def _dr_layout(a):
    """[1024, N] contraction-major -> [512, 2N] DoubleRow layout.

    Row r = j*128 + p holds contraction indices d = 256j + 128i + p in
    column halves i = 0, 1 (per 128-row block).
    """
    n = a.shape[1]
    return np.ascontiguousarray(
        a.reshape(NJ, 2, 128, n).transpose(0, 2, 1, 3).reshape(512, 2 * n)
    )


def _dr_layout_x(a):
    """[1024, T] -> [128, NJ*2*T] partition-major DoubleRow layout."""
    n = a.shape[1]
    return np.ascontiguousarray(
        a.reshape(NJ, 2, 128, n).transpose(2, 0, 1, 3).reshape(128, NJ * 2 * n)
    )


def _dr_layout_xhl(xh, xl):
    """Two [1024, T] hi/lo planes -> [128, NJ*2*2*T] interleaved layout.

    x_hl[p, j, i, h, t] holds contraction index d = 256j + 128i + p of the
    h-th (hi/lo) plane.
    """
    n = xh.shape[1]
    planes = [a.reshape(NJ, 2, 128, n).transpose(2, 0, 1, 3) for a in (xh, xl)]
    return np.ascontiguousarray(
        np.stack(planes, axis=3).reshape(128, NJ * 2 * 2 * n)
    )


def _prep_fast_inputs(x, wq, wk, wv, wo):
    ins = []
    # p_tile is [key, query]: allowed iff key <= query -> upper triangular
    em = np.triu(np.ones((128, 128), dtype=np.float32)).astype(bf)
    xq = []
    for b in range(B):
        xh, xl = _q8(x[b].T)  # [1024, 2048]
        xq.append(_dr_layout_xhl(xh, xl))
    wqs = []
    for g in range(2):
        d = {}
        for name, w in (("wq", wq), ("wk", wk)):
            wt = np.ascontiguousarray(w[g * FG:(g + 1) * FG, :].T) * WS  # [1024, 512]
            hi, lo = _q8(wt)
            d[name + "8"] = np.stack([_dr_layout(hi), _dr_layout(lo)])
        wvt = np.ascontiguousarray(wv[g * FG:(g + 1) * FG, :].T) * WS
        wvh, wvl = _q8(wvt)
        d["wv8"] = np.stack(
            [_dr_layout(a) for a in ((wvh,) if V_TERMS == 2 else (wvh, wvl))]
        )
        # wo in fp8 DR layout [p, a, r, o]: feature f = 256a + 128r + p,
        # pre-scaled by WS so hi sits in e4m3's sweet spot (undone in fin).
        wot = np.ascontiguousarray(wo[:, g * FG:(g + 1) * FG].T) * WS  # [512, 1024]
        woh, wol = _q8(wot)
        d["wo8h"] = np.ascontiguousarray(
            woh.reshape(2, 2, 128, DIM).transpose(2, 0, 1, 3).reshape(128, -1)
        )
        d["wo8l"] = np.ascontiguousarray(
            wol.reshape(2, 2, 128, DIM).transpose(2, 0, 1, 3).reshape(128, -1)
        )
        wqs.append(d)
    for i in range(8):
        b, g = i // 2, i % 2
        im = {"xhl": xq[b], "em": em}
        im.update(wqs[g])
        ins.append(im)
    return ins


# --------------------------------------------------------------------------
# legacy general/nomask path (baseline kernel, kept as fallback)
# --------------------------------------------------------------------------

TQ = 1024
NDC = DIM // 128


def _causal_sched():
    sched = []
    for a in (0, 1):
        for c in range(8 * a + 8):
            sched.append((a, c, (c - 8 * a) // 2))
    return sched


CAUSAL_SCHED = _causal_sched()


def _build_legacy(variant, reps=1):
    """variant: 'general' | 'nomask' (original baseline kernel)."""
    nc = bacc.Bacc("TRN2", target_bir_lowering=False, debug=False, num_devices=8)

    xT = nc.dram_tensor("xT", [DIM, T], BF16, kind="ExternalInput").ap()
    xqT = nc.dram_tensor("xqT", [DIM, TQ], BF16, kind="ExternalInput").ap()
    wqT = nc.dram_tensor("wqT", [DIM, DIM], BF16, kind="ExternalInput").ap()
    wkT = nc.dram_tensor("wkT", [DIM, DIM], BF16, kind="ExternalInput").ap()
    wvT = nc.dram_tensor("wvT", [DIM, DIM], BF16, kind="ExternalInput").ap()
    woT = nc.dram_tensor("woT", [DIM, DIM], BF16, kind="ExternalInput").ap()
    if variant == "general":
        em = nc.dram_tensor("em", [T, TQ], BF16, kind="ExternalInput").ap()
    else:
        em = None
    out = nc.dram_tensor("out", [TQ, DIM], F32, kind="ExternalOutput").ap()

    with tile.TileContext(nc) as tc:
      for _rep in range(reps):
        ctx = ExitStack()
        ctx.__enter__()
        Exp = mybir.ActivationFunctionType.Exp
        mult = mybir.AluOpType.mult

        qt_p = ctx.enter_context(tc.tile_pool(name="qt", bufs=1))
        kt_p = ctx.enter_context(tc.tile_pool(name="kt", bufs=1))
        va_p = ctx.enter_context(tc.tile_pool(name="va", bufs=1))
        misc_p = ctx.enter_context(tc.tile_pool(name="misc", bufs=1))
        psS = ctx.enter_context(tc.tile_pool(name="psS", bufs=2, space="PSUM"))
        psO = ctx.enter_context(tc.tile_pool(name="psO", bufs=1, space="PSUM"))
        psB = ctx.enter_context(tc.tile_pool(name="psB", bufs=1, space="PSUM"))
        psF = ctx.enter_context(tc.tile_pool(name="psF", bufs=1, space="PSUM"))

        qt_sb = [qt_p.tile([128, TQ], BF16, tag=f"qt{i}", name=f"qt{i}") for i in range(NDC)]
        kt_sb = [kt_p.tile([128, T], BF16, tag=f"kt{i}", name=f"kt{i}") for i in range(NDC)]
        va_sb = [va_p.tile([128, H * 65], BF16, tag=f"va{i}", name=f"va{i}") for i in range(NTC)]
        ones_sb = misc_p.tile([128, 64], F32, tag="ones", name="ones")
        nc.vector.memset(ones_sb[64:65, :], 1.0)

        with tc.tile_pool(name="xin", bufs=1) as x_p, tc.tile_pool(
            name="win", bufs=1
        ) as w_p:
            xt_sb = [x_p.tile([128, T], BF16, tag=f"xt{i}", name=f"xt{i}") for i in range(NDC)]
            xq_sb = [x_p.tile([128, TQ], BF16, tag=f"xq{i}", name=f"xq{i}") for i in range(NDC)]
            wq_sb = [w_p.tile([128, DIM], BF16, tag=f"wq{i}", name=f"wq{i}") for i in range(NDC)]
            wk_sb = [w_p.tile([128, DIM], BF16, tag=f"wk{i}", name=f"wk{i}") for i in range(NDC)]
            wv_sb = [w_p.tile([128, DIM], BF16, tag=f"wv{i}", name=f"wv{i}") for i in range(NDC)]
            for i in range(NDC):
                s = slice(i * 128, (i + 1) * 128)
                nc.sync.dma_start(wv_sb[i][:], wvT[s, :])
                nc.sync.dma_start(xt_sb[i][:], xT[s, :])
            for i in range(NDC):
                s = slice(i * 128, (i + 1) * 128)
                nc.sync.dma_start(wq_sb[i][:], wqT[s, :])
                nc.sync.dma_start(xq_sb[i][:], xqT[s, :])
            for i in range(NDC):
                s = slice(i * 128, (i + 1) * 128)
                nc.sync.dma_start(wk_sb[i][:], wkT[s, :])

            for c in range(NTC):
                v4 = va_sb[c][:].rearrange("p (q t x) -> p q t x", q=8, t=2)
                nc.vector.memset(v4[:, :, 0:2, 64:65], 1.0)

            for c in range(NTC):
                csl = slice(c * 128, (c + 1) * 128)
                ps = psS.tile([128, 1024], F32, tag="s", name="v_ps")
                for n in range(2):
                    nsl = slice(n * 512, (n + 1) * 512)
                    for dc in range(NDC):
                        nc.tensor.matmul(
                            ps[:, nsl],
                            xt_sb[dc][:, csl],
                            wv_sb[dc][:, nsl],
                            start=(dc == 0),
                            stop=(dc == NDC - 1),
                        )
                v4 = va_sb[c][:].rearrange("p (q t x) -> p q t x", q=8, t=2)
                s4 = ps[:].rearrange("p (q t x) -> p q t x", q=8, t=2)
                nc.vector.tensor_copy(v4[:, :, 0:2, 0:64], s4[:, :, 0:2, :])

            for oc in range(NDC):
                osl = slice(oc * 128, (oc + 1) * 128)
                ps = psS.tile([128, 1024], F32, tag="s", name="q_ps")
                for n in range(2):
                    nsl = slice(n * 512, (n + 1) * 512)
                    for dc in range(NDC):
                        nc.tensor.matmul(
                            ps[:, nsl],
                            wq_sb[dc][:, osl],
                            xq_sb[dc][:, nsl],
                            start=(dc == 0),
                            stop=(dc == NDC - 1),
                        )
                nc.scalar.copy(qt_sb[oc][:], ps[:])
                for m in range(2):
                    ps = psS.tile([128, 1024], F32, tag="s", name="k_ps")
                    for n in range(2):
                        nsl = slice((2 * m + n) * 512, (2 * m + n + 1) * 512)
                        psl = slice(n * 512, (n + 1) * 512)
                        for dc in range(NDC):
                            nc.tensor.matmul(
                                ps[:, psl],
                                wk_sb[dc][:, osl],
                                xt_sb[dc][:, nsl],
                                start=(dc == 0),
                                stop=(dc == NDC - 1),
                            )
                    nc.scalar.copy(
                        kt_sb[oc][:, m * 1024: (m + 1) * 1024], ps[:]
                    )

        em_p = ctx.enter_context(tc.tile_pool(name="em", bufs=1))
        wo_p = ctx.enter_context(tc.tile_pool(name="wo", bufs=1))
        otn_p = ctx.enter_context(tc.tile_pool(name="otn", bufs=1))
        p_p = ctx.enter_context(tc.tile_pool(name="pp", bufs=6))
        osb_p = ctx.enter_context(tc.tile_pool(name="osb", bufs=2))
        nrm_p = ctx.enter_context(tc.tile_pool(name="nrm", bufs=2))
        fin_p = ctx.enter_context(tc.tile_pool(name="fin", bufs=2))

        if variant == "general":
            em_sb = [em_p.tile([128, TQ], BF16, tag=f"em{i}", name=f"emt{i}") for i in range(NTC)]
            for c in range(NTC):
                nc.sync.dma_start(em_sb[c][:], em[c * 128: (c + 1) * 128, :])
        wo_sb = [wo_p.tile([128, DIM], BF16, tag=f"wo{i}", name=f"wot{i}") for i in range(NDC)]
        for i in range(NDC):
            nc.sync.dma_start(wo_sb[i][:], woT[i * 128: (i + 1) * 128, :])
        otn_sb = [otn_p.tile([128, TQ], BF16, tag=f"otn{i}", name=f"otn{i}") for i in range(NDC)]
        otn1_sb = [otn_p.tile([64, TQ], BF16, tag=f"otn1{i}", name=f"otn1{i}") for i in range(NDC)]

        for a in (0, 1):
            nA = NTC
            asl = slice(a * 512, (a + 1) * 512)
            for hp in range(NDC):
                o_pair = psO.tile([128, 1024], F32, tag="opair", name="o_pair")
                for c in range(nA):
                    csl = slice(c * 128, (c + 1) * 128)
                    s_ps = psS.tile([128, 1024], F32, tag="s", name="s_ps")
                    nc.tensor.matmul(
                        s_ps[:, 0:512],
                        kt_sb[hp][0:64, csl],
                        qt_sb[hp][0:64, asl],
                        start=True,
                        stop=True,
                    )
                    nc.tensor.matmul(
                        s_ps[:, 512:1024],
                        kt_sb[hp][64:128, csl],
                        qt_sb[hp][64:128, asl],
                        start=True,
                        stop=True,
                        tile_position=(64, 0),
                    )
                    p_pair = p_p.tile([128, 1024], BF16, tag="p", name="p_pair")
                    nc.scalar.activation(p_pair[:], s_ps[:], Exp)
                    if variant == "general":
                        for half in (0, 1):
                            psl = slice(half * 512, half * 512 + 512)
                            nc.vector.tensor_mul(
                                p_pair[:, psl], p_pair[:, psl], em_sb[c][:, asl]
                            )
                    va4 = va_sb[c][:].rearrange("p (q t x) -> p q t x", q=8, t=2)
                    nc.tensor.matmul(
                        o_pair[0:65, 0:512],
                        va4[:, hp, 0, :],
                        p_pair[:, 0:512],
                        start=(c == 0),
                        stop=(c == nA - 1),
                    )
                    nc.tensor.matmul(
                        o_pair[0:65, 512:1024],
                        va4[:, hp, 1, :],
                        p_pair[:, 512:1024],
                        start=(c == 0),
                        stop=(c == nA - 1),
                    )
                o_sb = osb_p.tile([128, 1024], F32, tag="osb", name="o_sb")
                nc.vector.tensor_copy(o_sb[:], o_pair[:])
                rec = nrm_p.tile([128, 1024], mybir.dt.float32r, tag="rec", name="rec")
                with nc.allow_low_precision(reason="f32r recip for denom broadcast"):
                    nc.vector.reciprocal(rec[64:65, 0:1024], o_sb[64:65, 0:1024])
                b0 = psB.tile([64, 512], F32, tag="b", name="b0")
                b1 = psB.tile([64, 512], F32, tag="b", name="b1")
                nc.tensor.matmul(
                    b0[:],
                    ones_sb[64:65, :].bitcast(mybir.dt.float32r),
                    rec[64:65, 0:512],
                    start=True,
                    stop=True,
                    tile_position=(64, 0),
                )
                nc.tensor.matmul(
                    b1[:],
                    ones_sb[64:65, :].bitcast(mybir.dt.float32r),
                    rec[64:65, 512:1024],
                    start=True,
                    stop=True,
                    tile_position=(64, 0),
                )
                nc.vector.tensor_tensor(
                    otn_sb[hp][0:64, asl], o_sb[0:64, 0:512], b0[:], mult
                )
                nc.vector.tensor_tensor(
                    otn1_sb[hp][0:64, asl], o_sb[0:64, 512:1024], b1[:], mult
                )
                nc.sync.dma_start(otn_sb[hp][64:128, asl], otn1_sb[hp][0:64, asl])

            for tt in range(4 * a, 4 * a + 4):
                tsl = slice((tt % 4) * 128 + a * 512, (tt % 4) * 128 + a * 512 + 128)
                fin = fin_p.tile([128, 1024], F32, tag="fin", name="fin")
                for n in range(2):
                    nsl = slice(n * 512, (n + 1) * 512)
                    f_ps = psF.tile([128, 512], F32, tag="fwo", name="f_ps")
                    for hp in range(NDC):
                        nc.tensor.matmul(
                            f_ps[:],
                            otn_sb[hp][:, tsl],
                            wo_sb[hp][:, nsl],
                            start=(hp == 0),
                            stop=(hp == NDC - 1),
                        )
                    nc.vector.tensor_copy(fin[:, nsl], f_ps[:])
                nc.sync.dma_start(out[tt * 128: (tt + 1) * 128, :], fin[:])

        ctx.__exit__(None, None, None)

    nc.compile()
    return nc


_NC_CACHE = {}


def _get_nc(variant):
    if variant not in _NC_CACHE:
        if variant == "causal":
            _NC_CACHE[variant] = _build_fast()
        else:
            _NC_CACHE[variant] = _build_legacy(variant)
    return _NC_CACHE[variant]


def _kernel_legacy(x, emask, wq, wk, wv, wo, variant):
    scale = 1.0 / np.sqrt(HD)
    wqT = np.ascontiguousarray((wq * scale).T).astype(bf)
    wkT = np.ascontiguousarray(wk.T).astype(bf)
    wvT = np.ascontiguousarray(wv.T).astype(bf)
    woT = np.ascontiguousarray(wo.T).astype(bf)

    perms = {}
    for p in (0, 1):
        perms[p] = np.concatenate(
            [np.arange(128) + 128 * j for j in range(p, 16, 2)]
        )

    in_maps = []
    for i in range(8):
        b, p = i // 2, i % 2
        perm = perms[p]
        xb = x[b]
        im = {
            "xT": np.ascontiguousarray(xb.T).astype(bf),
            "xqT": np.ascontiguousarray(xb[perm].T).astype(bf),
            "wqT": wqT,
            "wkT": wkT,
            "wvT": wvT,
            "woT": woT,
        }
        if variant == "general":
            im["em"] = np.ascontiguousarray(emask.T[:, perm]).astype(bf)
        in_maps.append(im)

    nc = _get_nc(variant)
    res = run_bass_kernel_spmd(nc, in_maps, core_ids=list(range(8)))

    out_full = np.empty((B, T, DIM), dtype=np.float32)
    for i in range(8):
        b, p = i // 2, i % 2
        out_full[b, perms[p]] = res.results[i]["out"]
    return out_full


def kernel(x, mask, wq, wk, wv, wo):
    x = np.asarray(x, dtype=np.float32)
    mask = np.asarray(mask, dtype=np.float32)
    wq = np.asarray(wq, dtype=np.float32)
    wk = np.asarray(wk, dtype=np.float32)
    wv = np.asarray(wv, dtype=np.float32)
    wo = np.asarray(wo, dtype=np.float32)

    m2 = mask[0, 0]
    emask = np.exp(np.minimum(m2, 60.0)).astype(np.float32)
    tril = np.tril(np.ones((T, T), dtype=np.float32))
    if np.array_equal(emask, tril):
        variant = "causal"
    elif np.all(m2 == 0.0):
        variant = "nomask"
    else:
        variant = "general"

    if variant != "causal":
        return _kernel_legacy(x, emask, wq, wk, wv, wo, variant)

    in_maps = _prep_fast_inputs(x, wq, wk, wv, wo)
    nc = _get_nc("causal")
    res = run_bass_kernel_spmd(nc, in_maps, core_ids=list(range(8)))

    out_full = np.empty((B, T, DIM), dtype=np.float32)
    for b in range(B):
        y0 = res.results[2 * b]["y"].astype(np.float32)
        y1 = res.results[2 * b + 1]["y"].astype(np.float32)
        out_full[b] = y0 + y1
    return out_full

